# revision 36
# baseline (speedup 1.0000x reference)
"""Trainium2 Bass kernel for nn_Attention_7919919694519.

Multi-head attention (B=2, L=2048, H=16, d=64) with two data-dependent masks:
  - V_len[b] masks HEADS h >= V_len[b]: the reference adds -1e12 to every
    score of those heads, which collapses (in fp32) to a uniform softmax, so
    the masked head's output is mean_k(v) = (mean_k V_seq) @ WV_h  (rank-1).
  - Q_len[b] zeroes output rows q >= Q_len[b].

Strategy (host-visible Q_len/V_len drive the work list):
  - Only unmasked heads with live q rows do real attention. Each unmasked
    head is a "job" needing ceil(Q_len[b]/512) q-chunks (the last chunk
    trimmed to a 128-multiple of its live rows). Jobs are packed into
    head-slots dealt across 8 NeuronCores (SPMD: same NEFF, different
    data); K/V are projected once per slot, q-chunks stream through.
    No collectives; host scatters/gathers.
  - Per chunk on device: project q (bf16 matmul), scores S^T[k,q] in PSUM,
    exp on ScalarE (PSUM->SBUF bf16), AV accumulation with a ones-column
    appended to v so softmax denominators fall out of the same matmuls,
    PE transpose, reciprocal+scale on VectorE, DMA out. Emission is
    software-pipelined across chunk-units so ScalarE (the bottleneck
    engine) never starves at unit boundaries.
  - Masked-head rank-1 content: device reduces V_seq over k (VectorE) and
    projects through WV/2048; host broadcasts rows (pure output assembly).
"""

import math
import numpy as np
import ml_dtypes

import concourse.bass as bass
import concourse.tile as tile
from concourse import bacc, mybir
from concourse.bass_utils import run_bass_kernel_spmd
from concourse.masks import make_identity
from contextlib import ExitStack

BF16 = ml_dtypes.bfloat16
N_CORES = 8
B_, L_, D_, H_ = 2, 2048, 64, 16
NQ = 512              # max q rows per chunk
KT = 16               # number of 128-row k tiles (L/128)
SPS_FD = 1536         # score-psum slot free dim (3 banks)

_cache = {}


def _chunk_plan(nq):
    """k-tile sub-chunk sizes. Each k-tile gets its own 512-f32 PSUM lane so
    matmul outputs never cross a bank boundary (chunk = 3 banks)."""
    cl = 3
    out = [cl] * (KT // cl)
    if KT % cl:
        out = [KT % cl] + out
    return out


def _unit_order(struct):
    """Round-robin (slot, position) order; index = DRAM row in qt/out."""
    order = []
    max_r = max(len(w) for w in struct)
    for r in range(max_r):
        for s in range(len(struct)):
            if r < len(struct[s]):
                order.append((s, r))
    return order


def _build(struct):
    """Build + compile the SPMD NEFF.

    struct: tuple of per-slot tuples of chunk q-widths, e.g.
    ((512, 512, 512, 128), (512, 512, 256))."""
    nc = bacc.Bacc("TRN2", target_bir_lowering=False, debug=False,
                   num_devices=N_CORES)
    dt = mybir.dt
    S = len(struct)
    # interleave slots round-robin so slot prologues overlap earlier slots'
    # compute and the kernel tail lands on the smallest chunk. unit index u
    # equals its DRAM row in qt/out (host uses the same ordering).
    units = [(s, r == 0, struct[s][r]) for s, r in _unit_order(struct)]
    NU = len(units)

    qt_d = nc.dram_tensor("qt", [NU, 64, NQ], dt.bfloat16, kind="ExternalInput").ap()
    kt_d = nc.dram_tensor("kt", [S, 64, L_], dt.bfloat16, kind="ExternalInput").ap()
    vt_d = nc.dram_tensor("vt", [S, 64, L_], dt.bfloat16, kind="ExternalInput").ap()
    w_d = nc.dram_tensor("w", [S, 64, 128], dt.bfloat16, kind="ExternalInput").ap()
    vtb_d = nc.dram_tensor("vtb", [B_, 64, L_], dt.float32, kind="ExternalInput").ap()
    wvm_d = nc.dram_tensor("wvm", [64, H_ * 64], dt.float32, kind="ExternalInput").ap()
    out_d = nc.dram_tensor("out", [NU, 64, NQ], dt.float32, kind="ExternalOutput").ap()
    mo_d = nc.dram_tensor("meanout", [128, 8, B_], dt.float32, kind="ExternalOutput").ap()

    with tile.TileContext(nc) as tc, ExitStack() as ctx:
        sbufs = max(2, S)   # all slots' K/V live concurrently (interleaved)
        inp = ctx.enter_context(tc.tile_pool(name="inp", bufs=sbufs))
        proj = ctx.enter_context(tc.tile_pool(name="proj", bufs=sbufs))
        expp = ctx.enter_context(tc.tile_pool(name="expp", bufs=3))
        ob = ctx.enter_context(tc.tile_pool(name="ob", bufs=3))
        single = ctx.enter_context(tc.tile_pool(name="single", bufs=1))
        ps_s = ctx.enter_context(tc.tile_pool(name="ps_s", bufs=2, space="PSUM"))
        ps_a = ctx.enter_context(tc.tile_pool(name="ps_a", bufs=2, space="PSUM"))

        ones1 = single.tile([1, 64], dt.bfloat16)
        nc.vector.memset(ones1[:], 1.0)

        st = [dict() for _ in range(NU)]
        slot_tiles = {}

        def slot_k_prologue(u, on_act=False):
            # w DMA + tile allocation. The whole QK weight product is folded
            # into the K side: ktTilde = (WK_h WQ_h^T / sqrt(d)) @ K^T once
            # per slot, so per-unit score matmuls read the raw qt DMA with no
            # per-unit projection chain. kt/vt DMAs are issued by slot_kv_dma
            # (after the first unit's qt DMA so the critical path leads the
            # DMA queue); the projection itself runs in slot_kproj.
            s, first, _ = units[u]
            if not first or s in slot_tiles:
                return
            w_sb = inp.tile([64, 128], dt.bfloat16, tag="w", name=f"w{s}")
            nc.sync.dma_start(w_sb[:], w_d[s])
            kt_sb = inp.tile([64, L_], dt.bfloat16, tag="kt", name=f"kt{s}")
            vt_sb = inp.tile([64, L_], dt.bfloat16, tag="vt", name=f"vt{s}")
            slot_tiles[s] = [w_sb, None, None, vt_sb, kt_sb]

        kprojd = set()

        def slot_kproj(u, on_act=False):
            s, first, _ = units[u]
            if not first or s in kprojd:
                return
            kprojd.add(s)
            w_sb, _, _, _, kt_sb = slot_tiles[s]
            ktT = proj.tile([64, L_], dt.bfloat16, tag="ktT", name=f"ktT{s}")
            for j in range(4):
                kps = ps_s.tile([64, 512], dt.float32, tag="ps", name=f"kps{s}_{j}")
                nc.tensor.matmul(kps[:], w_sb[:, 0:64],
                                 kt_sb[:, j * 512:(j + 1) * 512],
                                 start=True, stop=True)
                if on_act:
                    nc.scalar.copy(ktT[:, j * 512:(j + 1) * 512], kps[:])
                else:
                    nc.vector.tensor_copy(ktT[:, j * 512:(j + 1) * 512], kps[:])
            slot_tiles[s][1] = ktT

        kv_dmad = set()

        def slot_kv_dma(u):
            s, first, _ = units[u]
            if not first or s in kv_dmad:
                return
            kt_sb, vt_sb = slot_tiles[s][4], slot_tiles[s][3]
            nc.sync.dma_start(kt_sb[:], kt_d[s])
            nc.sync.dma_start(vt_sb[:], vt_d[s])
            kv_dmad.add(s)

        def slot_v_prologue(u):
            s, first, _ = units[u]
            if not first or slot_tiles[s][2] is not None:
                return
            w_sb, vt_sb = slot_tiles[s][0], slot_tiles[s][3]
            # v projection into [k=128, 16, 65] layout (col 64 = ones)
            v_sb = proj.tile([128, KT, 65], dt.bfloat16, tag="v_sb")
            for half in range(2):
                vps = ps_s.tile([128, 8 * 64], dt.float32, tag="ps")
                for j in range(8):
                    t = half * 8 + j
                    nc.tensor.matmul(vps[:, j * 64:(j + 1) * 64],
                                     vt_sb[:, t * 128:(t + 1) * 128],
                                     w_sb[:, 64:128], start=True, stop=True)
                nc.vector.tensor_copy(
                    v_sb[:, half * 8:(half + 1) * 8, 0:64],
                    vps[:].rearrange("p (t d) -> p t d", t=8))
            nc.vector.memset(v_sb[:, :, 64], 1.0)
            slot_tiles[s][2] = v_sb

        def unit_prologue(u):
            s, _, nq = units[u]
            d = st[u]
            d["s"] = s
            d["chunks"] = _chunk_plan(nq)
            d["offs"] = [sum(d["chunks"][:i]) for i in range(len(d["chunks"]) + 1)]
            d["nq"] = nq
            qt_sb = inp.tile([64, nq], dt.bfloat16, tag="qt", name=f"qt{u}")
            nc.sync.dma_start(qt_sb[:], qt_d[u][:, 0:nq])
            d["qTh"] = qt_sb
            d["sps"] = [None] * len(d["chunks"])
            d["ex"] = [None] * len(d["chunks"])

        def s_chunk(u, c):
            d = st[u]
            cl, nq = d["chunks"][c], d["nq"]
            sps = ps_s.tile([128, cl, 512], dt.float32, tag="ps", name=f"sps{u}_{c}")
            for j in range(cl):
                t = d["offs"][c] + j
                nc.tensor.matmul(sps[:, j, 0:nq],
                                 slot_tiles[d["s"]][1][:, t * 128:(t + 1) * 128],
                                 d["qTh"][:], start=True, stop=True)
            d["sps"][c] = sps

        def e_chunk(u, c):
            d = st[u]
            cl, nq = d["chunks"][c], d["nq"]
            ex = expp.tile([128, cl, nq], dt.bfloat16, tag="ex", name=f"ex{u}_{c}")
            nc.scalar.activation(ex[:], d["sps"][c][:, :, 0:nq],
                                 mybir.ActivationFunctionType.Exp)
            d["ex"][c] = ex

        def av_chunk(u, c):
            d = st[u]
            nq = d["nq"]
            v_sb = slot_tiles[d["s"]][2]
            if c == 0:
                d["av"] = ps_a.tile([65, nq], dt.float32, tag="pa", name=f"av{u}")
            for j in range(d["chunks"][c]):
                t = d["offs"][c] + j
                nc.tensor.matmul(d["av"][:], v_sb[:, t, :],
                                 d["ex"][c][:, j, :],
                                 start=(t == 0), stop=(t == KT - 1))

        def epilogue(u):
            # normalize in O^T layout: recip of the sums row, broadcast down
            # 64 partitions via a K=1 matmul, one multiply, one DMA. The
            # final [d, q] -> [q, d] transpose happens on the host (gather).
            d = st[u]
            nq = d["nq"]
            o_sb = ob.tile([65, nq], dt.float32, tag="o_sb", name=f"osb{u}")
            nc.vector.tensor_copy(o_sb[:], d["av"][:])
            rs = ob.tile([1, nq], dt.bfloat16, tag="rs", name=f"rs{u}")
            with nc.allow_low_precision(reason="softmax denominators are O(1e3); bf16 recip is plenty for the broadcast path"):
                nc.vector.reciprocal(rs[:], o_sb[64:65, :])
            rb = ps_a.tile([64, nq], dt.float32, tag="pa", name=f"rb{u}")
            nc.tensor.matmul(rb[:], ones1[:, 0:64], rs[:], start=True, stop=True)
            ot = ob.tile([64, nq], dt.float32, tag="ot", name=f"ot{u}")
            nc.vector.tensor_mul(ot[:], o_sb[0:64, :], rb[:])
            nc.sync.dma_start(out_d[u][:, 0:nq], ot[:])
            st[u] = None

        def mean_block():
            # masked-head rank-1 content: (sum_k V_seq) @ (WV/2048)
            wvm_sb = single.tile([64, H_ * 64], dt.float32)
            nc.sync.dma_start(wvm_sb[:], wvm_d[:])
            mvt = single.tile([64, B_], dt.float32)
            for b in range(B_):
                vtb_sb = inp.tile([64, L_], dt.float32, tag="vtb")
                nc.sync.dma_start(vtb_sb[:], vtb_d[b])
                nc.vector.reduce_sum(mvt[:, b:b + 1], vtb_sb[:],
                                     axis=mybir.AxisListType.X)
            mo_sb = single.tile([128, 8, B_], dt.float32)
            for c in range(8):
                mps = ps_a.tile([128, B_], dt.float32, tag="pa", name=f"mps{c}")
                nc.tensor.matmul(mps[:], wvm_sb[:, c * 128:(c + 1) * 128], mvt[:],
                                 start=True, stop=True)
                nc.vector.tensor_copy(mo_sb[:, c, :], mps[:])
            nc.sync.dma_start(mo_d[:], mo_sb[:])

        # software pipeline across chunk-units: the next unit's prologue and
        # first score chunk are emitted before this unit's AV tail/epilogue so
        # ScalarE never starves at unit boundaries.
        slot_k_prologue(0)
        unit_prologue(0)
        slot_kv_dma(0)
        slot_kproj(0, on_act=True)
        s_chunk(0, 0)
        e_chunk(0, 0)
        # prefetch every other slot's K/V DMAs + projection while unit 0 runs
        first_unit = {}
        for i, (s, first, _) in enumerate(units):
            if first:
                first_unit[s] = i
        for s in range(1, S):
            slot_k_prologue(first_unit[s])
            slot_kv_dma(first_unit[s])
            slot_kproj(first_unit[s])
        for u in range(NU):
            nch = len(st[u]["chunks"])
            for c in range(nch):
                if c + 1 < nch:
                    s_chunk(u, c + 1)
                    e_chunk(u, c + 1)
                    if c == max(0, nch - 2) and u + 1 < NU:
                        slot_k_prologue(u + 1)
                        slot_kv_dma(u + 1)
                        slot_kproj(u + 1)
                        unit_prologue(u + 1)
                        s_chunk(u + 1, 0)
                        e_chunk(u + 1, 0)
                if c == 0:
                    slot_v_prologue(u)
                av_chunk(u, c)
            epilogue(u)
            if u == max(0, NU // 2 - 1):
                mean_block()

    nc.compile()
    return nc


def _round128(x):
    return max(128, (x + 127) // 128 * 128)


def _plan(q_len, v_len, B, L, H):
    """Pack unmasked-head jobs into head-slots.

    Returns (struct, assign): struct[s] = tuple of chunk q-widths;
    assign[(core, s)] = (b, h) or None."""
    jobs = []
    for b in range(B):
        nq = min(max(q_len[b], 0), L)
        nh = min(max(v_len[b], 0), H)
        if nq <= 0:
            continue
        r = (nq + NQ - 1) // NQ
        for h in range(nh):
            jobs.append((r, nq, b, h))
    jobs.sort(key=lambda x: (-x[0], -x[1]))
    n_slots = max(1, (len(jobs) + N_CORES - 1) // N_CORES)
    struct = []
    assign = {}
    for s in range(n_slots):
        col = jobs[s * N_CORES:(s + 1) * N_CORES]
        rmax = col[0][0] if col else 1
        widths = []
        for r in range(rmax):
            live = max((min(NQ, nq - r * NQ) for (jr, nq, _, _) in col
                        if r < jr), default=64)
            widths.append(int(live))
        struct.append(tuple(widths))
        for c in range(N_CORES):
            assign[(c, s)] = (col[c][2], col[c][3]) if c < len(col) else None
    return tuple(struct), assign


def kernel(Q_seq, K_seq, V_seq, WQ, WK, WV, Q_len, V_len):
    Q_seq = np.asarray(Q_seq, dtype=np.float32)
    K_seq = np.asarray(K_seq, dtype=np.float32)
    V_seq = np.asarray(V_seq, dtype=np.float32)
    WQ = np.asarray(WQ, dtype=np.float32)
    WK = np.asarray(WK, dtype=np.float32)
    WV = np.asarray(WV, dtype=np.float32)
    q_len = [int(x) for x in np.asarray(Q_len).reshape(-1)]
    v_len = [int(x) for x in np.asarray(V_len).reshape(-1)]
    B, L, d = Q_seq.shape
    H = WQ.shape[1] // d
    scale = 1.0 / math.sqrt(d)

    struct, assign = _plan(q_len, v_len, B, L, H)
    S = len(struct)
    order = _unit_order(struct)
    row_of = {sr: i for i, sr in enumerate(order)}
    NU = len(order)

    if struct not in _cache:
        _cache[struct] = _build(struct)
    nc = _cache[struct]

    # host-side shard prep (transposes, bf16 casts, weight slicing)
    KTb = [np.ascontiguousarray(K_seq[b].T).astype(BF16) for b in range(B)]
    VTb = [np.ascontiguousarray(V_seq[b].T).astype(BF16) for b in range(B)]
    QT = [np.ascontiguousarray(Q_seq[b].T).astype(BF16) for b in range(B)]
    vtb = np.stack([V_seq[b].T for b in range(B)]).astype(np.float32)
    wvm = (WV / float(L)).astype(np.float32)

    in_maps = []
    for c in range(N_CORES):
        qt = np.zeros((NU, 64, NQ), dtype=BF16)
        kt = np.zeros((S, 64, L), dtype=BF16)
        vt = np.zeros((S, 64, L), dtype=BF16)
        w = np.zeros((S, 64, 128), dtype=BF16)
        for s in range(S):
            job = assign[(c, s)]
            if job is None:
                continue
            b, h = job
            kt[s] = KTb[b]
            vt[s] = VTb[b]
            wq_h = WQ[:, h * d:(h + 1) * d]
            wk_h = WK[:, h * d:(h + 1) * d]
            w[s, :, 0:64] = (wk_h @ wq_h.T * scale).astype(BF16)
            w[s, :, 64:128] = WV[:, h * d:(h + 1) * d].astype(BF16)
            for r, nqw in enumerate(struct[s]):
                q0 = min(r * NQ, L - nqw)
                qt[row_of[(s, r)], :, 0:nqw] = QT[b][:, q0:q0 + nqw]
        in_maps.append({"qt": qt, "kt": kt, "vt": vt, "w": w,
                        "vtb": vtb, "wvm": wvm})

    global _last_in_maps
    _last_in_maps = in_maps
    res = run_bass_kernel_spmd(nc, in_maps, core_ids=list(range(N_CORES)))
    results = res.results

    # gather
    out = np.zeros((B, L, H * d), dtype=np.float32)
    mo = results[0]["meanout"]  # [128, 8, B]
    mean_proj = np.transpose(mo, (2, 1, 0)).reshape(B, H * d)  # [B, H*d]
    for b in range(B):
        nq = min(max(q_len[b], 0), L)
        nh = min(max(v_len[b], 0), H)
        if nq > 0 and nh < H:
            out[b, :nq, nh * d:] = mean_proj[b, nh * d:][None, :]
    for (c, s), job in assign.items():
        if job is None:
            continue
        b, h = job
        nq = min(max(q_len[b], 0), L)
        for r, nqw in enumerate(struct[s]):
            q0 = min(r * NQ, L - nqw)
            lo, hi = q0, min(q0 + nqw, nq)
            if hi <= lo:
                continue
            out[b, lo:hi, h * d:(h + 1) * d] = \
                results[c]["out"][row_of[(s, r)], :, :hi - lo].T
    return out


# revision 39
# speedup vs baseline: 1.1662x; 1.1662x over previous
"""Trainium2 Bass kernel for nn_Attention_7919919694519.

Multi-head attention (B=2, L=2048, H=16, d=64) with two data-dependent masks:
  - V_len[b] masks HEADS h >= V_len[b]: the reference adds -1e12 to every
    score of those heads, which collapses (in fp32) to a uniform softmax, so
    the masked head's output is mean_k(v) = (mean_k V_seq) @ WV_h  (rank-1).
  - Q_len[b] zeroes output rows q >= Q_len[b].

Strategy (host-visible Q_len/V_len drive the work list):
  - Only unmasked heads with live q rows do real attention. Each unmasked
    head is a "job" needing ceil(Q_len[b]/512) q-chunks (the last chunk
    trimmed to a 128-multiple of its live rows). Jobs are packed into
    head-slots dealt across 8 NeuronCores (SPMD: same NEFF, different
    data); K/V are projected once per slot, q-chunks stream through.
    No collectives; host scatters/gathers.
  - Per chunk on device: project q (bf16 matmul), scores S^T[k,q] in PSUM,
    exp on ScalarE (PSUM->SBUF bf16), AV accumulation with a ones-column
    appended to v so softmax denominators fall out of the same matmuls,
    PE transpose, reciprocal+scale on VectorE, DMA out. Emission is
    software-pipelined across chunk-units so ScalarE (the bottleneck
    engine) never starves at unit boundaries.
  - Masked-head rank-1 content: device reduces V_seq over k (VectorE) and
    projects through WV/2048; host broadcasts rows (pure output assembly).
"""

import math
import numpy as np
import ml_dtypes

import concourse.bass as bass
import concourse.tile as tile
from concourse import bacc, mybir
from concourse.bass_utils import run_bass_kernel_spmd
from concourse.masks import make_identity
from contextlib import ExitStack

BF16 = ml_dtypes.bfloat16
N_CORES = 8
B_, L_, D_, H_ = 2, 2048, 64, 16
NQ = 512              # max q rows per chunk
KT = 16               # number of 128-row k tiles (L/128)
SPS_FD = 1536         # score-psum slot free dim (3 banks)

_cache = {}


def _per_bank(nq):
    """k-tiles packed per 512-f32 PSUM bank (outputs never cross a bank)."""
    return max(1, 512 // nq)


def _chunk_plan(nq):
    """k-tiles per score chunk: 2 banks per chunk, 3-deep buffered so score
    matmuls never wait on semaphore latency; narrow q-widths pack several
    k-tiles per bank to keep exp instruction count low."""
    cl = 2 * _per_bank(nq)
    out = [cl] * (KT // cl)
    if KT % cl:
        out = [KT % cl] + out
    return out


def _unit_order(struct):
    """Round-robin (slot, position) order; index = DRAM row in qt/out."""
    order = []
    max_r = max(len(w) for w in struct)
    for r in range(max_r):
        for s in range(len(struct)):
            if r < len(struct[s]):
                order.append((s, r))
    return order


def _build(struct):
    """Build + compile the SPMD NEFF.

    struct: tuple of per-slot tuples of chunk q-widths, e.g.
    ((512, 512, 512, 128), (512, 512, 256))."""
    nc = bacc.Bacc("TRN2", target_bir_lowering=False, debug=False,
                   num_devices=N_CORES)
    dt = mybir.dt
    S = len(struct)
    # interleave slots round-robin so slot prologues overlap earlier slots'
    # compute and the kernel tail lands on the smallest chunk. unit index u
    # equals its DRAM row in qt/out (host uses the same ordering).
    units = [(s, r == 0, struct[s][r]) for s, r in _unit_order(struct)]
    NU = len(units)

    qt_d = nc.dram_tensor("qt", [NU, 64, NQ], dt.bfloat16, kind="ExternalInput").ap()
    kt_d = nc.dram_tensor("kt", [S, 64, L_], dt.bfloat16, kind="ExternalInput").ap()
    vt_d = nc.dram_tensor("vt", [S, 64, L_], dt.bfloat16, kind="ExternalInput").ap()
    w_d = nc.dram_tensor("w", [S, 64, 128], dt.bfloat16, kind="ExternalInput").ap()
    vtb_d = nc.dram_tensor("vtb", [B_, 64, L_], dt.float32, kind="ExternalInput").ap()
    wvm_d = nc.dram_tensor("wvm", [64, H_ * 64], dt.float32, kind="ExternalInput").ap()
    out_d = nc.dram_tensor("out", [NU, 64, NQ], dt.float32, kind="ExternalOutput").ap()
    mo_d = nc.dram_tensor("meanout", [128, 8, B_], dt.float32, kind="ExternalOutput").ap()

    with tile.TileContext(nc) as tc, ExitStack() as ctx:
        sbufs = max(2, S)   # all slots' K/V live concurrently (interleaved)
        inp = ctx.enter_context(tc.tile_pool(name="inp", bufs=sbufs))
        proj = ctx.enter_context(tc.tile_pool(name="proj", bufs=sbufs))
        expp = ctx.enter_context(tc.tile_pool(name="expp", bufs=3))
        ob = ctx.enter_context(tc.tile_pool(name="ob", bufs=3))
        single = ctx.enter_context(tc.tile_pool(name="single", bufs=1))
        ps_s = ctx.enter_context(tc.tile_pool(name="ps_s", bufs=3, space="PSUM"))
        ps_a = ctx.enter_context(tc.tile_pool(name="ps_a", bufs=2, space="PSUM"))

        ones1 = single.tile([1, 64], dt.bfloat16)
        nc.vector.memset(ones1[:], 1.0)

        st = [dict() for _ in range(NU)]
        slot_tiles = {}

        def slot_k_prologue(u, on_act=False):
            # w DMA + tile allocation. The whole QK weight product is folded
            # into the K side: ktTilde = (WK_h WQ_h^T / sqrt(d)) @ K^T once
            # per slot, so per-unit score matmuls read the raw qt DMA with no
            # per-unit projection chain. kt/vt DMAs are issued by slot_kv_dma
            # (after the first unit's qt DMA so the critical path leads the
            # DMA queue); the projection itself runs in slot_kproj.
            s, first, _ = units[u]
            if not first or s in slot_tiles:
                return
            w_sb = inp.tile([64, 128], dt.bfloat16, tag="w", name=f"w{s}")
            nc.sync.dma_start(w_sb[:], w_d[s])
            kt_sb = inp.tile([64, L_], dt.bfloat16, tag="kt", name=f"kt{s}")
            vt_sb = inp.tile([64, L_], dt.bfloat16, tag="vt", name=f"vt{s}")
            slot_tiles[s] = [w_sb, None, None, vt_sb, kt_sb]

        kprojd = set()

        def slot_kproj(u, on_act=False):
            s, first, _ = units[u]
            if not first or s in kprojd:
                return
            kprojd.add(s)
            w_sb, _, _, _, kt_sb = slot_tiles[s]
            ktT = proj.tile([64, L_], dt.bfloat16, tag="ktT", name=f"ktT{s}")
            for j in range(4):
                kps = ps_s.tile([64, 512], dt.float32, tag="ps", name=f"kps{s}_{j}")
                nc.tensor.matmul(kps[:], w_sb[:, 0:64],
                                 kt_sb[:, j * 512:(j + 1) * 512],
                                 start=True, stop=True)
                if on_act and j == 0:
                    nc.scalar.copy(ktT[:, j * 512:(j + 1) * 512], kps[:])
                else:
                    nc.vector.tensor_copy(ktT[:, j * 512:(j + 1) * 512], kps[:])
            slot_tiles[s][1] = ktT

        kv_dmad = set()

        def slot_kv_dma(u):
            s, first, _ = units[u]
            if not first or s in kv_dmad:
                return
            kt_sb, vt_sb = slot_tiles[s][4], slot_tiles[s][3]
            nc.sync.dma_start(kt_sb[:], kt_d[s])
            nc.sync.dma_start(vt_sb[:], vt_d[s])
            kv_dmad.add(s)

        def slot_v_prologue(u):
            s, first, _ = units[u]
            if not first or slot_tiles[s][2] is not None:
                return
            w_sb, vt_sb = slot_tiles[s][0], slot_tiles[s][3]
            # v projection into [k=128, 16, 65] layout (col 64 = ones)
            v_sb = proj.tile([128, KT, 65], dt.bfloat16, tag="v_sb")
            for half in range(2):
                vps = ps_s.tile([128, 8 * 64], dt.float32, tag="ps")
                for j in range(8):
                    t = half * 8 + j
                    nc.tensor.matmul(vps[:, j * 64:(j + 1) * 64],
                                     vt_sb[:, t * 128:(t + 1) * 128],
                                     w_sb[:, 64:128], start=True, stop=True)
                nc.vector.tensor_copy(
                    v_sb[:, half * 8:(half + 1) * 8, 0:64],
                    vps[:].rearrange("p (t d) -> p t d", t=8))
            nc.vector.memset(v_sb[:, :, 64], 1.0)
            slot_tiles[s][2] = v_sb

        def unit_prologue(u):
            s, _, nq = units[u]
            d = st[u]
            d["s"] = s
            d["chunks"] = _chunk_plan(nq)
            d["offs"] = [sum(d["chunks"][:i]) for i in range(len(d["chunks"]) + 1)]
            d["nq"] = nq
            qt_sb = inp.tile([64, nq], dt.bfloat16, tag="qt", name=f"qt{u}")
            nc.sync.dma_start(qt_sb[:], qt_d[u][:, 0:nq])
            d["qTh"] = qt_sb
            d["sps"] = [None] * len(d["chunks"])
            d["ex"] = [None] * len(d["chunks"])

        def s_chunk(u, c):
            d = st[u]
            cl, nq = d["chunks"][c], d["nq"]
            pb = _per_bank(nq)
            nb = (cl + pb - 1) // pb
            sps = ps_s.tile([128, nb, pb, nq], dt.float32, tag="ps",
                            name=f"sps{u}_{c}",
                            padded_shape=[None, None, None, 512 // pb])
            for j in range(cl):
                t = d["offs"][c] + j
                nc.tensor.matmul(sps[:, j // pb, j % pb, :],
                                 slot_tiles[d["s"]][1][:, t * 128:(t + 1) * 128],
                                 d["qTh"][:], start=True, stop=True)
            d["sps"][c] = sps

        def e_chunk(u, c):
            d = st[u]
            cl, nq = d["chunks"][c], d["nq"]
            pb = _per_bank(nq)
            nb = (cl + pb - 1) // pb
            ex = expp.tile([128, nb, pb, nq], dt.bfloat16, tag="ex", name=f"ex{u}_{c}")
            nc.scalar.activation(ex[:], d["sps"][c][:],
                                 mybir.ActivationFunctionType.Exp)
            d["ex"][c] = ex

        def av_chunk(u, c):
            d = st[u]
            nq = d["nq"]
            v_sb = slot_tiles[d["s"]][2]
            if c == 0:
                d["av"] = ps_a.tile([65, nq], dt.float32, tag="pa", name=f"av{u}")
            pb = _per_bank(nq)
            for j in range(d["chunks"][c]):
                t = d["offs"][c] + j
                nc.tensor.matmul(d["av"][:], v_sb[:, t, :],
                                 d["ex"][c][:, j // pb, j % pb, :],
                                 start=(t == 0), stop=(t == KT - 1))

        def epilogue(u):
            # normalize in O^T layout: recip of the sums row, broadcast down
            # 64 partitions via a K=1 matmul, one multiply, one DMA. The
            # final [d, q] -> [q, d] transpose happens on the host (gather).
            d = st[u]
            nq = d["nq"]
            o_sb = ob.tile([65, nq], dt.float32, tag="o_sb", name=f"osb{u}")
            nc.vector.tensor_copy(o_sb[:], d["av"][:])
            rs = ob.tile([1, nq], dt.bfloat16, tag="rs", name=f"rs{u}")
            with nc.allow_low_precision(reason="softmax denominators are O(1e3); bf16 recip is plenty for the broadcast path"):
                nc.vector.reciprocal(rs[:], o_sb[64:65, :])
            rb = ps_a.tile([64, nq], dt.float32, tag="pa", name=f"rb{u}")
            nc.tensor.matmul(rb[:], ones1[:, 0:64], rs[:], start=True, stop=True)
            ot = ob.tile([64, nq], dt.float32, tag="ot", name=f"ot{u}")
            nc.vector.tensor_mul(ot[:], o_sb[0:64, :], rb[:])
            nc.sync.dma_start(out_d[u][:, 0:nq], ot[:])
            st[u] = None

        def mean_block():
            # masked-head rank-1 content: (sum_k V_seq) @ (WV/2048)
            wvm_sb = single.tile([64, H_ * 64], dt.float32)
            nc.sync.dma_start(wvm_sb[:], wvm_d[:])
            mvt = single.tile([64, B_], dt.float32)
            mvt4 = single.tile([64, B_, 4], dt.float32)
            for b in range(B_):
                vtb_sb = inp.tile([64, L_], dt.float32, tag="vtb")
                nc.sync.dma_start(vtb_sb[:], vtb_d[b])
                for j in range(4):
                    nc.vector.reduce_sum(mvt4[:, b, j:j + 1],
                                         vtb_sb[:, j * 512:(j + 1) * 512],
                                         axis=mybir.AxisListType.X)
                nc.vector.reduce_sum(mvt[:, b:b + 1], mvt4[:, b, :],
                                     axis=mybir.AxisListType.X)
            mo_sb = single.tile([128, 8, B_], dt.float32)
            for c in range(8):
                mps = ps_a.tile([128, B_], dt.float32, tag="pa", name=f"mps{c}")
                nc.tensor.matmul(mps[:], wvm_sb[:, c * 128:(c + 1) * 128], mvt[:],
                                 start=True, stop=True)
                nc.vector.tensor_copy(mo_sb[:, c, :], mps[:])
            nc.sync.dma_start(mo_d[:], mo_sb[:])

        # software pipeline across chunk-units: the next unit's prologue and
        # first score chunk are emitted before this unit's AV tail/epilogue so
        # ScalarE never starves at unit boundaries.
        slot_k_prologue(0)
        unit_prologue(0)
        slot_kv_dma(0)
        slot_kproj(0, on_act=True)
        s_chunk(0, 0)
        e_chunk(0, 0)
        # prefetch every other slot's K/V DMAs + projection while unit 0 runs
        first_unit = {}
        for i, (s, first, _) in enumerate(units):
            if first:
                first_unit[s] = i
        for s in range(1, S):
            slot_k_prologue(first_unit[s])
            slot_kv_dma(first_unit[s])
            slot_kproj(first_unit[s])
        for u in range(NU):
            nch = len(st[u]["chunks"])
            for c in range(nch):
                if c + 1 < nch:
                    s_chunk(u, c + 1)
                    e_chunk(u, c + 1)
                    if c == max(0, nch - 2) and u + 1 < NU:
                        slot_k_prologue(u + 1)
                        slot_kv_dma(u + 1)
                        slot_kproj(u + 1)
                        unit_prologue(u + 1)
                        s_chunk(u + 1, 0)
                        e_chunk(u + 1, 0)
                if c == 0:
                    slot_v_prologue(u)
                av_chunk(u, c)
            epilogue(u)
            if u == max(0, NU // 2 - 1):
                mean_block()

    nc.compile()
    return nc


def _round128(x):
    return max(128, (x + 127) // 128 * 128)


def _plan(q_len, v_len, B, L, H):
    """Pack unmasked-head jobs into head-slots.

    Returns (struct, assign): struct[s] = tuple of chunk q-widths;
    assign[(core, s)] = (b, h) or None."""
    jobs = []
    for b in range(B):
        nq = min(max(q_len[b], 0), L)
        nh = min(max(v_len[b], 0), H)
        if nq <= 0:
            continue
        r = (nq + NQ - 1) // NQ
        for h in range(nh):
            jobs.append((r, nq, b, h))
    jobs.sort(key=lambda x: (-x[0], -x[1]))
    n_slots = max(1, (len(jobs) + N_CORES - 1) // N_CORES)
    struct = []
    assign = {}
    for s in range(n_slots):
        col = jobs[s * N_CORES:(s + 1) * N_CORES]
        rmax = col[0][0] if col else 1
        widths = []
        for r in range(rmax):
            live = max((min(NQ, nq - r * NQ) for (jr, nq, _, _) in col
                        if r < jr), default=64)
            widths.append(int(live))
        struct.append(tuple(widths))
        for c in range(N_CORES):
            assign[(c, s)] = (col[c][2], col[c][3]) if c < len(col) else None
    return tuple(struct), assign


def kernel(Q_seq, K_seq, V_seq, WQ, WK, WV, Q_len, V_len):
    Q_seq = np.asarray(Q_seq, dtype=np.float32)
    K_seq = np.asarray(K_seq, dtype=np.float32)
    V_seq = np.asarray(V_seq, dtype=np.float32)
    WQ = np.asarray(WQ, dtype=np.float32)
    WK = np.asarray(WK, dtype=np.float32)
    WV = np.asarray(WV, dtype=np.float32)
    q_len = [int(x) for x in np.asarray(Q_len).reshape(-1)]
    v_len = [int(x) for x in np.asarray(V_len).reshape(-1)]
    B, L, d = Q_seq.shape
    H = WQ.shape[1] // d
    scale = 1.0 / math.sqrt(d)

    struct, assign = _plan(q_len, v_len, B, L, H)
    S = len(struct)
    order = _unit_order(struct)
    row_of = {sr: i for i, sr in enumerate(order)}
    NU = len(order)

    if struct not in _cache:
        _cache[struct] = _build(struct)
    nc = _cache[struct]

    # host-side shard prep (transposes, bf16 casts, weight slicing)
    KTb = [np.ascontiguousarray(K_seq[b].T).astype(BF16) for b in range(B)]
    VTb = [np.ascontiguousarray(V_seq[b].T).astype(BF16) for b in range(B)]
    QT = [np.ascontiguousarray(Q_seq[b].T).astype(BF16) for b in range(B)]
    vtb = np.stack([V_seq[b].T for b in range(B)]).astype(np.float32)
    wvm = (WV / float(L)).astype(np.float32)

    in_maps = []
    for c in range(N_CORES):
        qt = np.zeros((NU, 64, NQ), dtype=BF16)
        kt = np.zeros((S, 64, L), dtype=BF16)
        vt = np.zeros((S, 64, L), dtype=BF16)
        w = np.zeros((S, 64, 128), dtype=BF16)
        for s in range(S):
            job = assign[(c, s)]
            if job is None:
                continue
            b, h = job
            kt[s] = KTb[b]
            vt[s] = VTb[b]
            wq_h = WQ[:, h * d:(h + 1) * d]
            wk_h = WK[:, h * d:(h + 1) * d]
            w[s, :, 0:64] = (wk_h @ wq_h.T * scale).astype(BF16)
            w[s, :, 64:128] = WV[:, h * d:(h + 1) * d].astype(BF16)
            for r, nqw in enumerate(struct[s]):
                q0 = min(r * NQ, L - nqw)
                qt[row_of[(s, r)], :, 0:nqw] = QT[b][:, q0:q0 + nqw]
        in_maps.append({"qt": qt, "kt": kt, "vt": vt, "w": w,
                        "vtb": vtb, "wvm": wvm})

    global _last_in_maps
    _last_in_maps = in_maps
    res = run_bass_kernel_spmd(nc, in_maps, core_ids=list(range(N_CORES)))
    results = res.results

    # gather
    out = np.zeros((B, L, H * d), dtype=np.float32)
    mo = results[0]["meanout"]  # [128, 8, B]
    mean_proj = np.transpose(mo, (2, 1, 0)).reshape(B, H * d)  # [B, H*d]
    for b in range(B):
        nq = min(max(q_len[b], 0), L)
        nh = min(max(v_len[b], 0), H)
        if nq > 0 and nh < H:
            out[b, :nq, nh * d:] = mean_proj[b, nh * d:][None, :]
    for (c, s), job in assign.items():
        if job is None:
            continue
        b, h = job
        nq = min(max(q_len[b], 0), L)
        for r, nqw in enumerate(struct[s]):
            q0 = min(r * NQ, L - nqw)
            lo, hi = q0, min(q0 + nqw, nq)
            if hi <= lo:
                continue
            out[b, lo:hi, h * d:(h + 1) * d] = \
                results[c]["out"][row_of[(s, r)], :, :hi - lo].T
    return out


# revision 43
# speedup vs baseline: 1.2116x; 1.0389x over previous
"""Trainium2 Bass kernel for nn_Attention_7919919694519.

Multi-head attention (B=2, L=2048, H=16, d=64) with two data-dependent masks:
  - V_len[b] masks HEADS h >= V_len[b]: the reference adds -1e12 to every
    score of those heads, which collapses (in fp32) to a uniform softmax, so
    the masked head's output is mean_k(v) = (mean_k V_seq) @ WV_h  (rank-1).
  - Q_len[b] zeroes output rows q >= Q_len[b].

Strategy (host-visible Q_len/V_len drive the work list):
  - Only unmasked heads with live q rows do real attention. Each unmasked
    head is a "job" needing ceil(Q_len[b]/512) q-chunks (the last chunk
    trimmed to a 128-multiple of its live rows). Jobs are packed into
    head-slots dealt across 8 NeuronCores (SPMD: same NEFF, different
    data); K/V are projected once per slot, q-chunks stream through.
    No collectives; host scatters/gathers.
  - Per chunk on device: project q (bf16 matmul), scores S^T[k,q] in PSUM,
    exp on ScalarE (PSUM->SBUF bf16), AV accumulation with a ones-column
    appended to v so softmax denominators fall out of the same matmuls,
    PE transpose, reciprocal+scale on VectorE, DMA out. Emission is
    software-pipelined across chunk-units so ScalarE (the bottleneck
    engine) never starves at unit boundaries.
  - Masked-head rank-1 content: device reduces V_seq over k (VectorE) and
    projects through WV/2048; host broadcasts rows (pure output assembly).
"""

import math
import numpy as np
import ml_dtypes

import concourse.bass as bass
import concourse.tile as tile
from concourse import bacc, mybir
from concourse.bass_utils import run_bass_kernel_spmd
from concourse.masks import make_identity
from contextlib import ExitStack

BF16 = ml_dtypes.bfloat16
N_CORES = 8
B_, L_, D_, H_ = 2, 2048, 64, 16
NQ = 512              # max q rows per chunk
KT = 16               # number of 128-row k tiles (L/128)
SPS_FD = 1536         # score-psum slot free dim (3 banks)

_cache = {}


def _per_bank(nq):
    """k-tiles packed per 512-f32 PSUM bank (power of two so chunks always
    fill whole banks; outputs never cross a bank boundary)."""
    pb = 1
    while pb * 2 <= min(16, 512 // nq):
        pb *= 2
    return pb


def _chunk_plan(nq):
    """k-tiles per score chunk: 2 banks per chunk, 3-deep buffered so score
    matmuls never wait on semaphore latency; narrow q-widths pack several
    k-tiles per bank to keep exp instruction count low."""
    cl = 2 * _per_bank(nq)
    out = [cl] * (KT // cl)
    if KT % cl:
        out = [KT % cl] + out
    return out


def _unit_order(struct):
    """Round-robin (slot, position) order; index = DRAM row in qt/out."""
    order = []
    max_r = max(len(w) for w in struct)
    for r in range(max_r):
        for s in range(len(struct)):
            if r < len(struct[s]):
                order.append((s, r))
    return order


def _build(struct):
    """Build + compile the SPMD NEFF.

    struct: tuple of per-slot tuples of chunk q-widths, e.g.
    ((512, 512, 512, 128), (512, 512, 256))."""
    nc = bacc.Bacc("TRN2", target_bir_lowering=False, debug=False,
                   num_devices=N_CORES)
    dt = mybir.dt
    S = len(struct)
    # interleave slots round-robin so slot prologues overlap earlier slots'
    # compute and the kernel tail lands on the smallest chunk. unit index u
    # equals its DRAM row in qt/out (host uses the same ordering).
    units = [(s, r == 0, struct[s][r]) for s, r in _unit_order(struct)]
    NU = len(units)

    qt_d = nc.dram_tensor("qt", [NU, 64, NQ], dt.bfloat16, kind="ExternalInput").ap()
    kt_d = nc.dram_tensor("kt", [S, 64, L_], dt.bfloat16, kind="ExternalInput").ap()
    vt_d = nc.dram_tensor("vt", [S, 64, L_], dt.bfloat16, kind="ExternalInput").ap()
    w_d = nc.dram_tensor("w", [S, 64, 128], dt.bfloat16, kind="ExternalInput").ap()
    vtb_d = nc.dram_tensor("vtb", [B_, 64, L_], dt.float32, kind="ExternalInput").ap()
    wvm_d = nc.dram_tensor("wvm", [64, H_ * 64], dt.float32, kind="ExternalInput").ap()
    out_d = nc.dram_tensor("out", [NU, 64, NQ], dt.bfloat16, kind="ExternalOutput").ap()
    mo_d = nc.dram_tensor("meanout", [128, 8, B_], dt.float32, kind="ExternalOutput").ap()

    with tile.TileContext(nc) as tc, ExitStack() as ctx:
        sbufs = max(2, S)   # all slots' K/V live concurrently (interleaved)
        inp = ctx.enter_context(tc.tile_pool(name="inp", bufs=sbufs))
        proj = ctx.enter_context(tc.tile_pool(name="proj", bufs=sbufs))
        expp = ctx.enter_context(tc.tile_pool(name="expp", bufs=3))
        ob = ctx.enter_context(tc.tile_pool(name="ob", bufs=3))
        single = ctx.enter_context(tc.tile_pool(name="single", bufs=1))
        ps_s = ctx.enter_context(tc.tile_pool(name="ps_s", bufs=3, space="PSUM"))
        ps_a = ctx.enter_context(tc.tile_pool(name="ps_a", bufs=2, space="PSUM"))

        ones1 = single.tile([1, 64], dt.bfloat16)
        nc.vector.memset(ones1[:], 1.0)

        st = [dict() for _ in range(NU)]
        slot_tiles = {}

        def slot_k_prologue(u, on_act=False):
            # w DMA + tile allocation. The whole QK weight product is folded
            # into the K side: ktTilde = (WK_h WQ_h^T / sqrt(d)) @ K^T once
            # per slot, so per-unit score matmuls read the raw qt DMA with no
            # per-unit projection chain. kt/vt DMAs are issued by slot_kv_dma
            # (after the first unit's qt DMA so the critical path leads the
            # DMA queue); the projection itself runs in slot_kproj.
            s, first, _ = units[u]
            if not first or s in slot_tiles:
                return
            w_sb = inp.tile([64, 128], dt.bfloat16, tag="w", name=f"w{s}")
            nc.sync.dma_start(w_sb[:], w_d[s])
            kt_sb = inp.tile([64, L_], dt.bfloat16, tag="kt", name=f"kt{s}")
            vt_sb = inp.tile([64, L_], dt.bfloat16, tag="vt", name=f"vt{s}")
            slot_tiles[s] = [w_sb, None, None, vt_sb, kt_sb]

        kprojd = set()

        def slot_kproj(u, on_act=False):
            s, first, _ = units[u]
            if not first or s in kprojd:
                return
            kprojd.add(s)
            w_sb, _, _, _, kt_sb = slot_tiles[s]
            ktT = proj.tile([64, L_], dt.bfloat16, tag="ktT", name=f"ktT{s}")
            for j in range(4):
                kps = ps_s.tile([64, 512], dt.float32, tag="ps", name=f"kps{s}_{j}")
                nc.tensor.matmul(kps[:], w_sb[:, 0:64],
                                 kt_sb[:, j * 512:(j + 1) * 512],
                                 start=True, stop=True)
                if (on_act and j <= 1) or (not on_act and j == 0):
                    nc.scalar.copy(ktT[:, j * 512:(j + 1) * 512], kps[:])
                else:
                    nc.vector.tensor_copy(ktT[:, j * 512:(j + 1) * 512], kps[:])
            slot_tiles[s][1] = ktT

        kv_dmad = {}

        def slot_kv_dma(u, phase=2):
            s, first, _ = units[u]
            if not first:
                return
            done = kv_dmad.get(s, 0)
            kt_sb, vt_sb = slot_tiles[s][4], slot_tiles[s][3]
            if done < 1 and phase >= 0:
                nc.sync.dma_start(kt_sb[:, 0:512], kt_d[s][:, 0:512])
                kv_dmad[s] = 1
            if kv_dmad[s] < 2 and phase >= 1:
                nc.sync.dma_start(kt_sb[:, 512:], kt_d[s][:, 512:])
                nc.sync.dma_start(vt_sb[:], vt_d[s])
                kv_dmad[s] = 2

        def slot_v_prologue(u):
            s, first, _ = units[u]
            if not first or slot_tiles[s][2] is not None:
                return
            w_sb, vt_sb = slot_tiles[s][0], slot_tiles[s][3]
            # v projection into [k=128, 16, 65] layout (col 64 = ones)
            v_sb = proj.tile([128, KT, 65], dt.bfloat16, tag="v_sb")
            for half in range(2):
                vps = ps_s.tile([128, 8 * 64], dt.float32, tag="ps")
                for j in range(8):
                    t = half * 8 + j
                    nc.tensor.matmul(vps[:, j * 64:(j + 1) * 64],
                                     vt_sb[:, t * 128:(t + 1) * 128],
                                     w_sb[:, 64:128], start=True, stop=True)
                nc.vector.tensor_copy(
                    v_sb[:, half * 8:(half + 1) * 8, 0:64],
                    vps[:].rearrange("p (t d) -> p t d", t=8))
            nc.vector.memset(v_sb[:, :, 64], 1.0)
            slot_tiles[s][2] = v_sb

        def unit_prologue(u):
            s, _, nq = units[u]
            d = st[u]
            d["init"] = True
            d["s"] = s
            d["chunks"] = _chunk_plan(nq)
            d["offs"] = [sum(d["chunks"][:i]) for i in range(len(d["chunks"]) + 1)]
            d["nq"] = nq
            qt_sb = inp.tile([64, nq], dt.bfloat16, tag="qt", name=f"qt{u}")
            nc.sync.dma_start(qt_sb[:], qt_d[u][:, 0:nq])
            d["qTh"] = qt_sb
            d["sps"] = [None] * len(d["chunks"])
            d["ex"] = [None] * len(d["chunks"])

        def s_chunk(u, c):
            d = st[u]
            cl, nq = d["chunks"][c], d["nq"]
            pb = _per_bank(nq)
            nb = (cl + pb - 1) // pb
            sps = ps_s.tile([128, nb, pb, nq], dt.float32, tag="ps",
                            name=f"sps{u}_{c}",
                            padded_shape=[None, None, None, 512 // pb])
            for j in range(cl):
                t = d["offs"][c] + j
                nc.tensor.matmul(sps[:, j // pb, j % pb, :],
                                 slot_tiles[d["s"]][1][:, t * 128:(t + 1) * 128],
                                 d["qTh"][:], start=True, stop=True)
            d["sps"][c] = sps

        def e_chunk(u, c):
            d = st[u]
            cl, nq = d["chunks"][c], d["nq"]
            pb = _per_bank(nq)
            nb = (cl + pb - 1) // pb
            ex = expp.tile([128, nb, pb, nq], dt.bfloat16, tag="ex", name=f"ex{u}_{c}")
            nc.scalar.activation(ex[:], d["sps"][c][:],
                                 mybir.ActivationFunctionType.Exp)
            d["ex"][c] = ex

        def av_chunk(u, c):
            d = st[u]
            nq = d["nq"]
            v_sb = slot_tiles[d["s"]][2]
            if c == 0:
                d["av"] = ps_a.tile([65, nq], dt.float32, tag="pa", name=f"av{u}")
            pb = _per_bank(nq)
            for j in range(d["chunks"][c]):
                t = d["offs"][c] + j
                nc.tensor.matmul(d["av"][:], v_sb[:, t, :],
                                 d["ex"][c][:, j // pb, j % pb, :],
                                 start=(t == 0), stop=(t == KT - 1))

        def epilogue(u):
            # normalize in O^T layout: recip of the sums row, broadcast down
            # 64 partitions via a K=1 matmul, one multiply, one DMA. The
            # final [d, q] -> [q, d] transpose happens on the host (gather).
            d = st[u]
            nq = d["nq"]
            o_sb = ob.tile([65, nq], dt.float32, tag="o_sb", name=f"osb{u}")
            nc.vector.tensor_copy(o_sb[:], d["av"][:])
            rs = ob.tile([1, nq], dt.bfloat16, tag="rs", name=f"rs{u}")
            with nc.allow_low_precision(reason="softmax denominators are O(1e3); bf16 recip is plenty for the broadcast path"):
                nc.vector.reciprocal(rs[:], o_sb[64:65, :])
            rb = ps_a.tile([64, nq], dt.float32, tag="pa", name=f"rb{u}")
            nc.tensor.matmul(rb[:], ones1[:, 0:64], rs[:], start=True, stop=True)
            ot = ob.tile([64, nq], dt.bfloat16, tag="ot", name=f"ot{u}")
            with nc.allow_low_precision(reason="final output cast; 2e-2 rel-err budget"):
                nc.vector.tensor_mul(ot[:], o_sb[0:64, :], rb[:])
            nc.sync.dma_start(out_d[u][:, 0:nq], ot[:])
            st[u].clear()

        def mean_block():
            # masked-head rank-1 content: (sum_k V_seq) @ (WV/2048)
            wvm_sb = single.tile([64, H_ * 64], dt.float32)
            nc.sync.dma_start(wvm_sb[:], wvm_d[:])
            mvt = single.tile([64, B_], dt.float32)
            mvt4 = single.tile([64, B_, 4], dt.float32)
            for b in range(B_):
                vtb_sb = inp.tile([64, L_], dt.float32, tag="vtb")
                nc.sync.dma_start(vtb_sb[:], vtb_d[b])
                for j in range(4):
                    nc.vector.reduce_sum(mvt4[:, b, j:j + 1],
                                         vtb_sb[:, j * 512:(j + 1) * 512],
                                         axis=mybir.AxisListType.X)
                nc.vector.reduce_sum(mvt[:, b:b + 1], mvt4[:, b, :],
                                     axis=mybir.AxisListType.X)
            mo_sb = single.tile([128, 8, B_], dt.float32)
            mps = ps_a.tile([128, 8, B_], dt.float32, tag="pa", name="mps")
            for c in range(8):
                nc.tensor.matmul(mps[:, c, :], wvm_sb[:, c * 128:(c + 1) * 128],
                                 mvt[:], start=True, stop=True)
            nc.vector.tensor_copy(mo_sb[:], mps[:])
            nc.sync.dma_start(mo_d[:], mo_sb[:])

        # software pipeline across chunk-units: the next unit's prologue and
        # first score chunk are emitted before this unit's AV tail/epilogue so
        # ScalarE never starves at unit boundaries.
        slot_k_prologue(0)
        slot_kv_dma(0, phase=0)
        unit_prologue(0)
        slot_kv_dma(0, phase=1)
        slot_kproj(0, on_act=True)
        s_chunk(0, 0)
        e_chunk(0, 0)
        # prefetch every other slot's K/V DMAs + projection while unit 0 runs
        first_unit = {}
        for i, (s, first, _) in enumerate(units):
            if first:
                first_unit[s] = i
        for s in range(1, S):
            slot_k_prologue(first_unit[s])
            slot_kv_dma(first_unit[s])
            slot_kproj(first_unit[s])
        def prefetch_next(u1):
            if u1 >= NU or st[u1].get("init"):
                return
            slot_k_prologue(u1)
            slot_kv_dma(u1)
            slot_kproj(u1)
            unit_prologue(u1)
            s_chunk(u1, 0)
            e_chunk(u1, 0)

        for u in range(NU):
            nch = len(st[u]["chunks"])
            for c in range(nch):
                if c + 1 < nch:
                    s_chunk(u, c + 1)
                    e_chunk(u, c + 1)
                    if c == max(0, nch - 2):
                        prefetch_next(u + 1)
                elif u + 1 < NU:
                    prefetch_next(u + 1)
                if c == 0:
                    slot_v_prologue(u)
                av_chunk(u, c)
            epilogue(u)
            if u == max(0, NU // 2 - 1):
                mean_block()

    nc.compile()
    return nc


def _round128(x):
    return max(128, (x + 127) // 128 * 128)


def _plan(q_len, v_len, B, L, H):
    """Pack unmasked-head jobs into head-slots.

    Returns (struct, assign): struct[s] = tuple of chunk q-widths;
    assign[(core, s)] = (b, h) or None."""
    jobs = []
    for b in range(B):
        nq = min(max(q_len[b], 0), L)
        nh = min(max(v_len[b], 0), H)
        if nq <= 0:
            continue
        r = (nq + NQ - 1) // NQ
        for h in range(nh):
            jobs.append((r, nq, b, h))
    jobs.sort(key=lambda x: (-x[0], -x[1]))
    n_slots = max(1, (len(jobs) + N_CORES - 1) // N_CORES)
    struct = []
    assign = {}
    for s in range(n_slots):
        col = jobs[s * N_CORES:(s + 1) * N_CORES]
        rmax = col[0][0] if col else 1
        widths = []
        for r in range(rmax):
            live = max((min(NQ, nq - r * NQ) for (jr, nq, _, _) in col
                        if r < jr), default=64)
            widths.append(int(live))
        struct.append(tuple(widths))
        for c in range(N_CORES):
            assign[(c, s)] = (col[c][2], col[c][3]) if c < len(col) else None
    return tuple(struct), assign


def kernel(Q_seq, K_seq, V_seq, WQ, WK, WV, Q_len, V_len):
    Q_seq = np.asarray(Q_seq, dtype=np.float32)
    K_seq = np.asarray(K_seq, dtype=np.float32)
    V_seq = np.asarray(V_seq, dtype=np.float32)
    WQ = np.asarray(WQ, dtype=np.float32)
    WK = np.asarray(WK, dtype=np.float32)
    WV = np.asarray(WV, dtype=np.float32)
    q_len = [int(x) for x in np.asarray(Q_len).reshape(-1)]
    v_len = [int(x) for x in np.asarray(V_len).reshape(-1)]
    B, L, d = Q_seq.shape
    H = WQ.shape[1] // d
    scale = 1.0 / math.sqrt(d)

    struct, assign = _plan(q_len, v_len, B, L, H)
    S = len(struct)
    order = _unit_order(struct)
    row_of = {sr: i for i, sr in enumerate(order)}
    NU = len(order)

    if struct not in _cache:
        _cache[struct] = _build(struct)
    nc = _cache[struct]

    # host-side shard prep (transposes, bf16 casts, weight slicing)
    KTb = [np.ascontiguousarray(K_seq[b].T).astype(BF16) for b in range(B)]
    VTb = [np.ascontiguousarray(V_seq[b].T).astype(BF16) for b in range(B)]
    QT = [np.ascontiguousarray(Q_seq[b].T).astype(BF16) for b in range(B)]
    vtb = np.stack([V_seq[b].T for b in range(B)]).astype(np.float32)
    wvm = (WV / float(L)).astype(np.float32)

    in_maps = []
    for c in range(N_CORES):
        qt = np.zeros((NU, 64, NQ), dtype=BF16)
        kt = np.zeros((S, 64, L), dtype=BF16)
        vt = np.zeros((S, 64, L), dtype=BF16)
        w = np.zeros((S, 64, 128), dtype=BF16)
        for s in range(S):
            job = assign[(c, s)]
            if job is None:
                continue
            b, h = job
            kt[s] = KTb[b]
            vt[s] = VTb[b]
            wq_h = WQ[:, h * d:(h + 1) * d]
            wk_h = WK[:, h * d:(h + 1) * d]
            w[s, :, 0:64] = (wk_h @ wq_h.T * scale).astype(BF16)
            w[s, :, 64:128] = WV[:, h * d:(h + 1) * d].astype(BF16)
            for r, nqw in enumerate(struct[s]):
                q0 = min(r * NQ, L - nqw)
                qt[row_of[(s, r)], :, 0:nqw] = QT[b][:, q0:q0 + nqw]
        in_maps.append({"qt": qt, "kt": kt, "vt": vt, "w": w,
                        "vtb": vtb, "wvm": wvm})

    global _last_in_maps
    _last_in_maps = in_maps
    res = run_bass_kernel_spmd(nc, in_maps, core_ids=list(range(N_CORES)))
    results = res.results

    # gather
    out = np.zeros((B, L, H * d), dtype=np.float32)
    mo = results[0]["meanout"]  # [128, 8, B]
    mean_proj = np.transpose(mo, (2, 1, 0)).reshape(B, H * d)  # [B, H*d]
    for b in range(B):
        nq = min(max(q_len[b], 0), L)
        nh = min(max(v_len[b], 0), H)
        if nq > 0 and nh < H:
            out[b, :nq, nh * d:] = mean_proj[b, nh * d:][None, :]
    for (c, s), job in assign.items():
        if job is None:
            continue
        b, h = job
        nq = min(max(q_len[b], 0), L)
        for r, nqw in enumerate(struct[s]):
            q0 = min(r * NQ, L - nqw)
            lo, hi = q0, min(q0 + nqw, nq)
            if hi <= lo:
                continue
            out[b, lo:hi, h * d:(h + 1) * d] = \
                results[c]["out"][row_of[(s, r)], :, :hi - lo].T.astype(np.float32)
    return out


# revision 46
# speedup vs baseline: 125.1001x; 103.2522x over previous
"""Trainium2 Bass kernel for nn_Attention_7919919694519.

Multi-head attention (B=2, L=2048, H=16, d=64) with two data-dependent masks:
  - V_len[b] masks HEADS h >= V_len[b]: the reference adds -1e12 to every
    score of those heads, which collapses (in fp32) to a uniform softmax, so
    the masked head's output is mean_k(v) = (mean_k V_seq) @ WV_h  (rank-1).
  - Q_len[b] zeroes output rows q >= Q_len[b].

Strategy (host-visible Q_len/V_len drive the work list):
  - Only unmasked heads with live q rows do real attention. Each unmasked
    head is a "job" needing ceil(Q_len[b]/512) q-chunks (the last chunk
    trimmed to a 128-multiple of its live rows). Jobs are packed into
    head-slots dealt across 8 NeuronCores (SPMD: same NEFF, different
    data); K/V are projected once per slot, q-chunks stream through.
    No collectives; host scatters/gathers.
  - Per chunk on device: project q (bf16 matmul), scores S^T[k,q] in PSUM,
    exp on ScalarE (PSUM->SBUF bf16), AV accumulation with a ones-column
    appended to v so softmax denominators fall out of the same matmuls,
    PE transpose, reciprocal+scale on VectorE, DMA out. Emission is
    software-pipelined across chunk-units so ScalarE (the bottleneck
    engine) never starves at unit boundaries.
  - Masked-head rank-1 content: device reduces V_seq over k (VectorE) and
    projects through WV/2048; host broadcasts rows (pure output assembly).
"""

import math
import numpy as np
import ml_dtypes

import concourse.tile as tile
from concourse import bacc, mybir
from concourse.bass_utils import run_bass_kernel_spmd
from contextlib import ExitStack

BF16 = ml_dtypes.bfloat16
N_CORES = 8
B_, L_, D_, H_ = 2, 2048, 64, 16
NQ = 512              # max q rows per chunk
KT = 16               # number of 128-row k tiles (L/128)
SPS_FD = 1536         # score-psum slot free dim (3 banks)

_cache = {}


def _per_bank(nq):
    """k-tiles packed per 512-f32 PSUM bank (power of two so chunks always
    fill whole banks; outputs never cross a bank boundary)."""
    pb = 1
    while pb * 2 <= min(16, 512 // nq):
        pb *= 2
    return pb


def _chunk_plan(nq):
    """k-tiles per score chunk: 2 banks per chunk, 3-deep buffered so score
    matmuls never wait on semaphore latency; narrow q-widths pack several
    k-tiles per bank to keep exp instruction count low."""
    cl = 2 * _per_bank(nq)
    out = [cl] * (KT // cl)
    if KT % cl:
        out = [KT % cl] + out
    return out


def _unit_order(struct):
    """Round-robin (slot, position) order; index = DRAM row in qt/out."""
    order = []
    max_r = max(len(w) for w in struct)
    for r in range(max_r):
        for s in range(len(struct)):
            if r < len(struct[s]):
                order.append((s, r))
    return order


def _build(struct):
    """Build + compile the SPMD NEFF.

    struct: tuple of per-slot tuples of chunk q-widths, e.g.
    ((512, 512, 512, 128), (512, 512, 256))."""
    nc = bacc.Bacc("TRN2", target_bir_lowering=False, debug=False,
                   num_devices=N_CORES)
    dt = mybir.dt
    S = len(struct)
    # interleave slots round-robin so slot prologues overlap earlier slots'
    # compute and the kernel tail lands on the smallest chunk. unit index u
    # equals its DRAM row in qt/out (host uses the same ordering).
    units = [(s, r == 0, struct[s][r]) for s, r in _unit_order(struct)]
    NU = len(units)

    qt_d = nc.dram_tensor("qt", [NU, 64, NQ], dt.bfloat16, kind="ExternalInput").ap()
    kt_d = nc.dram_tensor("kt", [S, 64, L_], dt.bfloat16, kind="ExternalInput").ap()
    vt_d = nc.dram_tensor("vt", [S, 64, L_], dt.bfloat16, kind="ExternalInput").ap()
    w_d = nc.dram_tensor("w", [S, 64, 128], dt.bfloat16, kind="ExternalInput").ap()
    vtb_d = nc.dram_tensor("vtb", [B_, 64, L_], dt.float32, kind="ExternalInput").ap()
    wvm_d = nc.dram_tensor("wvm", [64, H_ * 64], dt.float32, kind="ExternalInput").ap()
    out_d = nc.dram_tensor("out", [NU, 64, NQ], dt.bfloat16, kind="ExternalOutput").ap()
    mo_d = nc.dram_tensor("meanout", [128, 8, B_], dt.float32, kind="ExternalOutput").ap()

    with tile.TileContext(nc) as tc, ExitStack() as ctx:
        sbufs = max(2, S)   # all slots' K/V live concurrently (interleaved)
        inp = ctx.enter_context(tc.tile_pool(name="inp", bufs=sbufs))
        proj = ctx.enter_context(tc.tile_pool(name="proj", bufs=sbufs))
        expp = ctx.enter_context(tc.tile_pool(name="expp", bufs=3))
        ob = ctx.enter_context(tc.tile_pool(name="ob", bufs=3))
        single = ctx.enter_context(tc.tile_pool(name="single", bufs=1))
        ps_s = ctx.enter_context(tc.tile_pool(name="ps_s", bufs=3, space="PSUM"))
        ps_a = ctx.enter_context(tc.tile_pool(name="ps_a", bufs=2, space="PSUM"))

        ones1 = single.tile([1, 64], dt.bfloat16)
        nc.vector.memset(ones1[:], 1.0)

        st = [dict() for _ in range(NU)]
        slot_tiles = {}

        def slot_k_prologue(u):
            # w DMA + tile allocation. The whole QK weight product is folded
            # into the K side: ktTilde = (WK_h WQ_h^T / sqrt(d)) @ K^T once
            # per slot, so per-unit score matmuls read the raw qt DMA with no
            # per-unit projection chain. kt/vt DMAs are issued by slot_kv_dma
            # (after the first unit's qt DMA so the critical path leads the
            # DMA queue); the projection itself runs in slot_kproj.
            s, first, _ = units[u]
            if not first or s in slot_tiles:
                return
            w_sb = inp.tile([64, 128], dt.bfloat16, tag="w", name=f"w{s}")
            nc.sync.dma_start(w_sb[:], w_d[s])
            kt_sb = inp.tile([64, L_], dt.bfloat16, tag="kt", name=f"kt{s}")
            vt_sb = inp.tile([64, L_], dt.bfloat16, tag="vt", name=f"vt{s}")
            slot_tiles[s] = [w_sb, None, None, vt_sb, kt_sb]

        kprojd = set()

        def slot_kproj(u):
            s, first, _ = units[u]
            if not first or s in kprojd:
                return
            kprojd.add(s)
            w_sb, _, _, _, kt_sb = slot_tiles[s]
            ktT = proj.tile([64, L_], dt.bfloat16, tag="ktT", name=f"ktT{s}")
            for j in range(4):
                kps = ps_s.tile([64, 512], dt.float32, tag="ps", name=f"kps{s}_{j}")
                nc.tensor.matmul(kps[:], w_sb[:, 0:64],
                                 kt_sb[:, j * 512:(j + 1) * 512],
                                 start=True, stop=True)
                nc.vector.tensor_copy(ktT[:, j * 512:(j + 1) * 512], kps[:])
            slot_tiles[s][1] = ktT

        kv_dmad = {}

        def slot_kv_dma(u, phase=2):
            s, first, _ = units[u]
            if not first:
                return
            done = kv_dmad.get(s, 0)
            kt_sb, vt_sb = slot_tiles[s][4], slot_tiles[s][3]
            if done < 1 and phase >= 0:
                nc.sync.dma_start(kt_sb[:, 0:512], kt_d[s][:, 0:512])
                kv_dmad[s] = 1
            if kv_dmad[s] < 2 and phase >= 1:
                nc.sync.dma_start(kt_sb[:, 512:], kt_d[s][:, 512:])
                nc.sync.dma_start(vt_sb[:], vt_d[s])
                kv_dmad[s] = 2

        def slot_v_prologue(u):
            s, first, _ = units[u]
            if not first or slot_tiles[s][2] is not None:
                return
            w_sb, vt_sb = slot_tiles[s][0], slot_tiles[s][3]
            # v projection into [k=128, 16, 65] layout (col 64 = ones)
            v_sb = proj.tile([128, KT, 65], dt.bfloat16, tag="v_sb")
            for half in range(2):
                vps = ps_s.tile([128, 8 * 64], dt.float32, tag="ps")
                for j in range(8):
                    t = half * 8 + j
                    nc.tensor.matmul(vps[:, j * 64:(j + 1) * 64],
                                     vt_sb[:, t * 128:(t + 1) * 128],
                                     w_sb[:, 64:128], start=True, stop=True)
                nc.vector.tensor_copy(
                    v_sb[:, half * 8:(half + 1) * 8, 0:64],
                    vps[:].rearrange("p (t d) -> p t d", t=8))
            nc.vector.memset(v_sb[:, :, 64], 1.0)
            slot_tiles[s][2] = v_sb

        def unit_prologue(u):
            s, _, nq = units[u]
            d = st[u]
            d["init"] = True
            d["s"] = s
            d["chunks"] = _chunk_plan(nq)
            d["offs"] = [sum(d["chunks"][:i]) for i in range(len(d["chunks"]) + 1)]
            d["nq"] = nq
            qt_sb = inp.tile([64, nq], dt.bfloat16, tag="qt", name=f"qt{u}")
            nc.sync.dma_start(qt_sb[:], qt_d[u][:, 0:nq])
            d["qTh"] = qt_sb
            d["sps"] = [None] * len(d["chunks"])
            d["ex"] = [None] * len(d["chunks"])

        def s_chunk(u, c):
            d = st[u]
            cl, nq = d["chunks"][c], d["nq"]
            pb = _per_bank(nq)
            nb = (cl + pb - 1) // pb
            sps = ps_s.tile([128, nb, pb, nq], dt.float32, tag="ps",
                            name=f"sps{u}_{c}",
                            padded_shape=[None, None, None, 512 // pb])
            for j in range(cl):
                t = d["offs"][c] + j
                nc.tensor.matmul(sps[:, j // pb, j % pb, :],
                                 slot_tiles[d["s"]][1][:, t * 128:(t + 1) * 128],
                                 d["qTh"][:], start=True, stop=True)
            d["sps"][c] = sps

        def e_chunk(u, c):
            d = st[u]
            cl, nq = d["chunks"][c], d["nq"]
            pb = _per_bank(nq)
            nb = (cl + pb - 1) // pb
            ex = expp.tile([128, nb, pb, nq], dt.bfloat16, tag="ex", name=f"ex{u}_{c}")
            nc.scalar.activation(ex[:], d["sps"][c][:],
                                 mybir.ActivationFunctionType.Exp)
            d["ex"][c] = ex

        def av_chunk(u, c):
            d = st[u]
            nq = d["nq"]
            v_sb = slot_tiles[d["s"]][2]
            if c == 0:
                d["av"] = ps_a.tile([65, nq], dt.float32, tag="pa", name=f"av{u}")
            pb = _per_bank(nq)
            for j in range(d["chunks"][c]):
                t = d["offs"][c] + j
                nc.tensor.matmul(d["av"][:], v_sb[:, t, :],
                                 d["ex"][c][:, j // pb, j % pb, :],
                                 start=(t == 0), stop=(t == KT - 1))

        def epilogue(u):
            # normalize in O^T layout: recip of the sums row, broadcast down
            # 64 partitions via a K=1 matmul, one multiply, one DMA. The
            # final [d, q] -> [q, d] transpose happens on the host (gather).
            d = st[u]
            nq = d["nq"]
            o_sb = ob.tile([65, nq], dt.float32, tag="o_sb", name=f"osb{u}")
            nc.vector.tensor_copy(o_sb[:], d["av"][:])
            rs = ob.tile([1, nq], dt.bfloat16, tag="rs", name=f"rs{u}")
            with nc.allow_low_precision(reason="softmax denominators are O(1e3); bf16 recip is plenty for the broadcast path"):
                nc.vector.reciprocal(rs[:], o_sb[64:65, :])
            rb = ps_a.tile([64, nq], dt.float32, tag="pa", name=f"rb{u}")
            nc.tensor.matmul(rb[:], ones1[:, 0:64], rs[:], start=True, stop=True)
            ot = ob.tile([64, nq], dt.bfloat16, tag="ot", name=f"ot{u}")
            with nc.allow_low_precision(reason="final output cast; 2e-2 rel-err budget"):
                nc.vector.tensor_mul(ot[:], o_sb[0:64, :], rb[:])
            nc.sync.dma_start(out_d[u][:, 0:nq], ot[:])
            st[u].clear()

        def mean_block():
            # masked-head rank-1 content: (sum_k V_seq) @ (WV/2048)
            wvm_sb = single.tile([64, H_ * 64], dt.float32)
            nc.sync.dma_start(wvm_sb[:], wvm_d[:])
            mvt = single.tile([64, B_], dt.float32)
            mvt4 = single.tile([64, B_, 4], dt.float32)
            for b in range(B_):
                vtb_sb = inp.tile([64, L_], dt.float32, tag="vtb")
                nc.sync.dma_start(vtb_sb[:], vtb_d[b])
                for j in range(4):
                    nc.vector.reduce_sum(mvt4[:, b, j:j + 1],
                                         vtb_sb[:, j * 512:(j + 1) * 512],
                                         axis=mybir.AxisListType.X)
                nc.vector.reduce_sum(mvt[:, b:b + 1], mvt4[:, b, :],
                                     axis=mybir.AxisListType.X)
            mo_sb = single.tile([128, 8, B_], dt.float32)
            mps = ps_a.tile([128, 8, B_], dt.float32, tag="pa", name="mps")
            for c in range(8):
                nc.tensor.matmul(mps[:, c, :], wvm_sb[:, c * 128:(c + 1) * 128],
                                 mvt[:], start=True, stop=True)
            nc.vector.tensor_copy(mo_sb[:], mps[:])
            nc.sync.dma_start(mo_d[:], mo_sb[:])

        # software pipeline across chunk-units: the next unit's prologue and
        # first score chunk are emitted before this unit's AV tail/epilogue so
        # ScalarE never starves at unit boundaries.
        slot_k_prologue(0)
        slot_kv_dma(0, phase=0)
        unit_prologue(0)
        slot_kv_dma(0, phase=1)
        slot_kproj(0)
        s_chunk(0, 0)
        e_chunk(0, 0)
        # prefetch every other slot's K/V DMAs + projection while unit 0 runs
        first_unit = {}
        for i, (s, first, _) in enumerate(units):
            if first:
                first_unit[s] = i
        for s in range(1, S):
            slot_k_prologue(first_unit[s])
            slot_kv_dma(first_unit[s])
            slot_kproj(first_unit[s])
        def prefetch_next(u1):
            if u1 >= NU or st[u1].get("init"):
                return
            slot_k_prologue(u1)
            slot_kv_dma(u1)
            slot_kproj(u1)
            unit_prologue(u1)
            s_chunk(u1, 0)
            e_chunk(u1, 0)

        for u in range(NU):
            nch = len(st[u]["chunks"])
            for c in range(nch):
                if c + 1 < nch:
                    s_chunk(u, c + 1)
                    e_chunk(u, c + 1)
                    if c == max(0, nch - 2):
                        prefetch_next(u + 1)
                elif u + 1 < NU:
                    prefetch_next(u + 1)
                if c == 0:
                    slot_v_prologue(u)
                av_chunk(u, c)
            epilogue(u)
            if u == max(0, NU // 2 - 1):
                mean_block()

    nc.compile()
    return nc


def _plan(q_len, v_len, B, L, H):
    """Pack unmasked-head jobs into head-slots.

    Returns (struct, assign): struct[s] = tuple of chunk q-widths;
    assign[(core, s)] = (b, h) or None."""
    jobs = []
    for b in range(B):
        nq = min(max(q_len[b], 0), L)
        nh = min(max(v_len[b], 0), H)
        if nq <= 0:
            continue
        r = (nq + NQ - 1) // NQ
        for h in range(nh):
            jobs.append((r, nq, b, h))
    jobs.sort(key=lambda x: (-x[0], -x[1]))
    n_slots = max(1, (len(jobs) + N_CORES - 1) // N_CORES)
    struct = []
    assign = {}
    for s in range(n_slots):
        col = jobs[s * N_CORES:(s + 1) * N_CORES]
        rmax = col[0][0] if col else 1
        widths = []
        for r in range(rmax):
            live = max((min(NQ, nq - r * NQ) for (jr, nq, _, _) in col
                        if r < jr), default=64)
            widths.append(int(live))
        struct.append(tuple(widths))
        for c in range(N_CORES):
            assign[(c, s)] = (col[c][2], col[c][3]) if c < len(col) else None
    return tuple(struct), assign


def kernel(Q_seq, K_seq, V_seq, WQ, WK, WV, Q_len, V_len):
    Q_seq = np.asarray(Q_seq, dtype=np.float32)
    K_seq = np.asarray(K_seq, dtype=np.float32)
    V_seq = np.asarray(V_seq, dtype=np.float32)
    WQ = np.asarray(WQ, dtype=np.float32)
    WK = np.asarray(WK, dtype=np.float32)
    WV = np.asarray(WV, dtype=np.float32)
    q_len = [int(x) for x in np.asarray(Q_len).reshape(-1)]
    v_len = [int(x) for x in np.asarray(V_len).reshape(-1)]
    B, L, d = Q_seq.shape
    H = WQ.shape[1] // d
    scale = 1.0 / math.sqrt(d)

    struct, assign = _plan(q_len, v_len, B, L, H)
    S = len(struct)
    order = _unit_order(struct)
    row_of = {sr: i for i, sr in enumerate(order)}
    NU = len(order)

    if struct not in _cache:
        _cache[struct] = _build(struct)
    nc = _cache[struct]

    # host-side shard prep (transposes, bf16 casts, weight slicing)
    KTb = [np.ascontiguousarray(K_seq[b].T).astype(BF16) for b in range(B)]
    VTb = [np.ascontiguousarray(V_seq[b].T).astype(BF16) for b in range(B)]
    QT = [np.ascontiguousarray(Q_seq[b].T).astype(BF16) for b in range(B)]
    vtb = np.stack([V_seq[b].T for b in range(B)]).astype(np.float32)
    wvm = (WV / float(L)).astype(np.float32)

    in_maps = []
    for c in range(N_CORES):
        qt = np.zeros((NU, 64, NQ), dtype=BF16)
        kt = np.zeros((S, 64, L), dtype=BF16)
        vt = np.zeros((S, 64, L), dtype=BF16)
        w = np.zeros((S, 64, 128), dtype=BF16)
        for s in range(S):
            job = assign[(c, s)]
            if job is None:
                continue
            b, h = job
            kt[s] = KTb[b]
            vt[s] = VTb[b]
            wq_h = WQ[:, h * d:(h + 1) * d]
            wk_h = WK[:, h * d:(h + 1) * d]
            w[s, :, 0:64] = (wk_h @ wq_h.T * scale).astype(BF16)
            w[s, :, 64:128] = WV[:, h * d:(h + 1) * d].astype(BF16)
            for r, nqw in enumerate(struct[s]):
                q0 = min(r * NQ, L - nqw)
                qt[row_of[(s, r)], :, 0:nqw] = QT[b][:, q0:q0 + nqw]
        in_maps.append({"qt": qt, "kt": kt, "vt": vt, "w": w,
                        "vtb": vtb, "wvm": wvm})

    global _last_in_maps
    _last_in_maps = in_maps
    res = run_bass_kernel_spmd(nc, in_maps, core_ids=list(range(N_CORES)))
    results = res.results

    # gather
    out = np.zeros((B, L, H * d), dtype=np.float32)
    mo = results[0]["meanout"]  # [128, 8, B]
    mean_proj = np.transpose(mo, (2, 1, 0)).reshape(B, H * d)  # [B, H*d]
    for b in range(B):
        nq = min(max(q_len[b], 0), L)
        nh = min(max(v_len[b], 0), H)
        if nq > 0 and nh < H:
            out[b, :nq, nh * d:] = mean_proj[b, nh * d:][None, :]
    for (c, s), job in assign.items():
        if job is None:
            continue
        b, h = job
        nq = min(max(q_len[b], 0), L)
        for r, nqw in enumerate(struct[s]):
            q0 = min(r * NQ, L - nqw)
            lo, hi = q0, min(q0 + nqw, nq)
            if hi <= lo:
                continue
            out[b, lo:hi, h * d:(h + 1) * d] = \
                results[c]["out"][row_of[(s, r)], :, :hi - lo].T.astype(np.float32)
    return out


# revision 51
# speedup vs baseline: 126.9658x; 1.0149x over previous
"""Trainium2 Bass kernel for nn_Attention_7919919694519.

Multi-head attention (B=2, L=2048, H=16, d=64) with two data-dependent masks:
  - V_len[b] masks HEADS h >= V_len[b]: the reference adds -1e12 to every
    score of those heads, which collapses (in fp32) to a uniform softmax, so
    the masked head's output is mean_k(v) = (mean_k V_seq) @ WV_h  (rank-1).
  - Q_len[b] zeroes output rows q >= Q_len[b].

Strategy (host-visible Q_len/V_len drive the work list):
  - Only unmasked heads with live q rows do real attention. Each unmasked
    head is a "job" needing ceil(Q_len[b]/512) q-chunks (the last chunk
    trimmed to its live rows). Jobs are packed into head-slots dealt across
    8 NeuronCores (SPMD: same NEFF, different data); slots are interleaved
    round-robin. No collectives; host scatters/gathers.
  - The QK weight product is reassociated: S = Q (WQ WK^T/sqrt(d)) K^T, so
    one per-slot projection ktTilde = (WK_h WQ_h^T/sqrt(d)) @ K^T replaces
    both q- and k-projections; score matmuls read the raw q DMA directly.
  - Per chunk on device: scores S^T[k,q] in bank-aligned PSUM lanes, exp on
    ScalarE (PSUM->SBUF bf16, the bottleneck engine), AV accumulation with
    a ones-column appended to v so softmax denominators fall out of the
    same matmuls, then reciprocal (VectorE) + ones-matmul broadcast +
    multiply, single bf16 DMA out in O^T layout (host transposes during
    gather). Emission is software-pipelined across chunk-units with 3-deep
    score-PSUM buffering so ScalarE never starves.
  - Masked-head rank-1 content: device reduces V_seq over k (VectorE) and
    projects through WV/2048; host broadcasts rows (pure output assembly).
"""

import math
import numpy as np
import ml_dtypes

import concourse.tile as tile
from concourse import bacc, mybir
from concourse.bass_utils import run_bass_kernel_spmd
from contextlib import ExitStack

BF16 = ml_dtypes.bfloat16
N_CORES = 8
B_, L_, D_, H_ = 2, 2048, 64, 16
NQ = 512              # max q rows per chunk
KT = 16               # number of 128-row k tiles (L/128)
SPS_FD = 1536         # score-psum slot free dim (3 banks)

_cache = {}


def _per_bank(nq):
    """k-tiles packed per 512-f32 PSUM bank (power of two so chunks always
    fill whole banks; outputs never cross a bank boundary)."""
    pb = 1
    while pb * 2 <= min(16, 512 // nq):
        pb *= 2
    return pb


def _chunk_plan(nq):
    """k-tiles per score chunk: 2 banks per chunk, 3-deep buffered so score
    matmuls never wait on semaphore latency; narrow q-widths pack several
    k-tiles per bank to keep exp instruction count low."""
    cl = 2 * _per_bank(nq)
    out = [cl] * (KT // cl)
    if KT % cl:
        out = [KT % cl] + out
    return out


def _unit_order(struct):
    """Round-robin (slot, position) order; index = DRAM row in qt/out."""
    order = []
    max_r = max(len(w) for w in struct)
    for r in range(max_r):
        for s in range(len(struct)):
            if r < len(struct[s]):
                order.append((s, r))
    return order


def _build(struct):
    """Build + compile the SPMD NEFF.

    struct: tuple of per-slot tuples of chunk q-widths, e.g.
    ((512, 512, 512, 128), (512, 512, 256))."""
    nc = bacc.Bacc("TRN2", target_bir_lowering=False, debug=False,
                   num_devices=N_CORES)
    dt = mybir.dt
    S = len(struct)
    # interleave slots round-robin so slot prologues overlap earlier slots'
    # compute and the kernel tail lands on the smallest chunk. unit index u
    # equals its DRAM row in qt/out (host uses the same ordering).
    units = [(s, r == 0, struct[s][r]) for s, r in _unit_order(struct)]
    NU = len(units)

    qt_d = nc.dram_tensor("qt", [NU, 64, NQ], dt.bfloat16, kind="ExternalInput").ap()
    kt_d = nc.dram_tensor("kt", [S, 64, L_], dt.bfloat16, kind="ExternalInput").ap()
    vt_d = nc.dram_tensor("vt", [S, 64, L_], dt.bfloat16, kind="ExternalInput").ap()
    w_d = nc.dram_tensor("w", [S, 64, 128], dt.bfloat16, kind="ExternalInput").ap()
    vtb_d = nc.dram_tensor("vtb", [B_, 64, L_], dt.float32, kind="ExternalInput").ap()
    wvm_d = nc.dram_tensor("wvm", [64, H_ * 64], dt.float32, kind="ExternalInput").ap()
    out_d = nc.dram_tensor("out", [NU, 64, NQ], dt.bfloat16, kind="ExternalOutput").ap()
    mo_d = nc.dram_tensor("meanout", [128, 8, B_], dt.float32, kind="ExternalOutput").ap()

    with tile.TileContext(nc) as tc, ExitStack() as ctx:
        sbufs = max(2, S)   # all slots' K/V live concurrently (interleaved)
        inp = ctx.enter_context(tc.tile_pool(name="inp", bufs=sbufs))
        proj = ctx.enter_context(tc.tile_pool(name="proj", bufs=sbufs))
        expp = ctx.enter_context(tc.tile_pool(name="expp", bufs=3))
        ob = ctx.enter_context(tc.tile_pool(name="ob", bufs=3))
        single = ctx.enter_context(tc.tile_pool(name="single", bufs=1))
        ps_s = ctx.enter_context(tc.tile_pool(name="ps_s", bufs=3, space="PSUM"))
        ps_a = ctx.enter_context(tc.tile_pool(name="ps_a", bufs=2, space="PSUM"))

        ones1 = single.tile([1, 64], dt.bfloat16)
        nc.vector.memset(ones1[:], 1.0)

        st = [dict() for _ in range(NU)]
        slot_tiles = {}

        def slot_k_prologue(u):
            # w DMA + tile allocation. The whole QK weight product is folded
            # into the K side: ktTilde = (WK_h WQ_h^T / sqrt(d)) @ K^T once
            # per slot, so per-unit score matmuls read the raw qt DMA with no
            # per-unit projection chain. kt/vt DMAs are issued by slot_kv_dma
            # (after the first unit's qt DMA so the critical path leads the
            # DMA queue); the projection itself runs in slot_kproj.
            s, first, _ = units[u]
            if not first or s in slot_tiles:
                return
            w_sb = inp.tile([64, 128], dt.bfloat16, tag="w", name=f"w{s}")
            nc.gpsimd.dma_start(w_sb[:], w_d[s])
            kt_sb = inp.tile([64, L_], dt.bfloat16, tag="kt", name=f"kt{s}")
            vt_sb = inp.tile([64, L_], dt.bfloat16, tag="vt", name=f"vt{s}")
            slot_tiles[s] = [w_sb, None, None, vt_sb, kt_sb]

        kprojd = set()

        def slot_kproj(u):
            s, first, _ = units[u]
            if not first or s in kprojd:
                return
            kprojd.add(s)
            w_sb, _, _, _, kt_sb = slot_tiles[s]
            ktT = proj.tile([64, L_], dt.bfloat16, tag="ktT", name=f"ktT{s}")
            for j in range(4):
                kps = ps_s.tile([64, 512], dt.float32, tag="ps", name=f"kps{s}_{j}")
                nc.tensor.matmul(kps[:], w_sb[:, 0:64],
                                 kt_sb[:, j * 512:(j + 1) * 512],
                                 start=True, stop=True)
                nc.vector.tensor_copy(ktT[:, j * 512:(j + 1) * 512], kps[:])
            slot_tiles[s][1] = ktT

        kv_dmad = {}

        def slot_kv_dma(u, phase=2):
            s, first, _ = units[u]
            if not first:
                return
            done = kv_dmad.get(s, 0)
            kt_sb, vt_sb = slot_tiles[s][4], slot_tiles[s][3]
            if done < 1 and phase >= 0:
                nc.sync.dma_start(kt_sb[:, 0:512], kt_d[s][:, 0:512])
                kv_dmad[s] = 1
            if kv_dmad[s] < 2 and phase >= 1:
                nc.sync.dma_start(kt_sb[:, 512:], kt_d[s][:, 512:])
                nc.gpsimd.dma_start(vt_sb[:], vt_d[s])
                kv_dmad[s] = 2

        def slot_v_prologue(u):
            s, first, _ = units[u]
            if not first or slot_tiles[s][2] is not None:
                return
            w_sb, vt_sb = slot_tiles[s][0], slot_tiles[s][3]
            # v projection into [k=128, 16, 65] layout (col 64 = ones)
            v_sb = proj.tile([128, KT, 65], dt.bfloat16, tag="v_sb")
            for half in range(2):
                vps = ps_s.tile([128, 8 * 64], dt.float32, tag="ps")
                for j in range(8):
                    t = half * 8 + j
                    nc.tensor.matmul(vps[:, j * 64:(j + 1) * 64],
                                     vt_sb[:, t * 128:(t + 1) * 128],
                                     w_sb[:, 64:128], start=True, stop=True)
                nc.vector.tensor_copy(
                    v_sb[:, half * 8:(half + 1) * 8, 0:64],
                    vps[:].rearrange("p (t d) -> p t d", t=8))
            nc.vector.memset(v_sb[:, :, 64], 1.0)
            slot_tiles[s][2] = v_sb

        def unit_prologue(u):
            s, _, nq = units[u]
            d = st[u]
            d["init"] = True
            d["s"] = s
            d["chunks"] = _chunk_plan(nq)
            d["offs"] = [sum(d["chunks"][:i]) for i in range(len(d["chunks"]) + 1)]
            d["nq"] = nq
            qt_sb = inp.tile([64, nq], dt.bfloat16, tag="qt", name=f"qt{u}")
            nc.sync.dma_start(qt_sb[:], qt_d[u][:, 0:nq])
            d["qTh"] = qt_sb
            d["sps"] = [None] * len(d["chunks"])
            d["ex"] = [None] * len(d["chunks"])

        def s_chunk(u, c):
            d = st[u]
            cl, nq = d["chunks"][c], d["nq"]
            pb = _per_bank(nq)
            nb = (cl + pb - 1) // pb
            sps = ps_s.tile([128, nb, pb, nq], dt.float32, tag="ps",
                            name=f"sps{u}_{c}",
                            padded_shape=[None, None, None, 512 // pb])
            for j in range(cl):
                t = d["offs"][c] + j
                nc.tensor.matmul(sps[:, j // pb, j % pb, :],
                                 slot_tiles[d["s"]][1][:, t * 128:(t + 1) * 128],
                                 d["qTh"][:], start=True, stop=True)
            d["sps"][c] = sps

        def e_chunk(u, c):
            d = st[u]
            cl, nq = d["chunks"][c], d["nq"]
            pb = _per_bank(nq)
            nb = (cl + pb - 1) // pb
            ex = expp.tile([128, nb, pb, nq], dt.bfloat16, tag="ex", name=f"ex{u}_{c}")
            nc.scalar.activation(ex[:], d["sps"][c][:],
                                 mybir.ActivationFunctionType.Exp)
            d["ex"][c] = ex

        def av_chunk(u, c):
            d = st[u]
            nq = d["nq"]
            v_sb = slot_tiles[d["s"]][2]
            if c == 0:
                d["av"] = ps_a.tile([65, nq], dt.float32, tag="pa", name=f"av{u}")
            pb = _per_bank(nq)
            for j in range(d["chunks"][c]):
                t = d["offs"][c] + j
                nc.tensor.matmul(d["av"][:], v_sb[:, t, :],
                                 d["ex"][c][:, j // pb, j % pb, :],
                                 start=(t == 0), stop=(t == KT - 1))

        def epilogue(u):
            # normalize in O^T layout: recip of the sums row, broadcast down
            # 64 partitions via a K=1 matmul, one multiply, one DMA. The
            # final [d, q] -> [q, d] transpose happens on the host (gather).
            d = st[u]
            nq = d["nq"]
            rs = ob.tile([1, nq], dt.bfloat16, tag="rs", name=f"rs{u}")
            with nc.allow_low_precision(reason="softmax denominators are O(1e3); bf16 recip is plenty for the broadcast path"):
                nc.vector.reciprocal(rs[:], d["av"][64:65, :])
            o_sb = ob.tile([64, nq], dt.float32, tag="o_sb", name=f"osb{u}")
            nc.vector.tensor_copy(o_sb[:], d["av"][0:64, :])
            rb = ps_a.tile([64, nq], dt.float32, tag="pa", name=f"rb{u}")
            nc.tensor.matmul(rb[:], ones1[:, 0:64], rs[:], start=True, stop=True)
            ot = ob.tile([64, nq], dt.bfloat16, tag="ot", name=f"ot{u}")
            with nc.allow_low_precision(reason="final output cast; 2e-2 rel-err budget"):
                nc.vector.tensor_mul(ot[:], o_sb[:], rb[:])
            nc.sync.dma_start(out_d[u][:, 0:nq], ot[:])
            st[u].clear()

        def mean_block():
            # masked-head rank-1 content: (sum_k V_seq) @ (WV/2048)
            wvm_sb = single.tile([64, H_ * 64], dt.float32)
            nc.sync.dma_start(wvm_sb[:], wvm_d[:])
            mvt = single.tile([64, B_], dt.float32)
            mvt4 = single.tile([64, B_, 4], dt.float32)
            for b in range(B_):
                vtb_sb = inp.tile([64, L_], dt.float32, tag="vtb")
                nc.sync.dma_start(vtb_sb[:], vtb_d[b])
                for j in range(4):
                    nc.vector.reduce_sum(mvt4[:, b, j:j + 1],
                                         vtb_sb[:, j * 512:(j + 1) * 512],
                                         axis=mybir.AxisListType.X)
                nc.vector.reduce_sum(mvt[:, b:b + 1], mvt4[:, b, :],
                                     axis=mybir.AxisListType.X)
            mo_sb = single.tile([128, 8, B_], dt.float32)
            mps = ps_a.tile([128, 8, B_], dt.float32, tag="pa", name="mps")
            for c in range(8):
                nc.tensor.matmul(mps[:, c, :], wvm_sb[:, c * 128:(c + 1) * 128],
                                 mvt[:], start=True, stop=True)
            nc.vector.tensor_copy(mo_sb[:], mps[:])
            nc.sync.dma_start(mo_d[:], mo_sb[:])

        # software pipeline across chunk-units: the next unit's prologue and
        # first score chunk are emitted before this unit's AV tail/epilogue so
        # ScalarE never starves at unit boundaries.
        slot_k_prologue(0)
        slot_kv_dma(0, phase=0)
        unit_prologue(0)
        slot_kv_dma(0, phase=1)
        slot_kproj(0)
        s_chunk(0, 0)
        e_chunk(0, 0)
        # prefetch every other slot's K/V DMAs + projection while unit 0 runs
        first_unit = {}
        for i, (s, first, _) in enumerate(units):
            if first:
                first_unit[s] = i
        for s in range(1, S):
            slot_k_prologue(first_unit[s])
            slot_kv_dma(first_unit[s])
            slot_kproj(first_unit[s])
        def prefetch_next(u1):
            if u1 >= NU or st[u1].get("init"):
                return
            slot_k_prologue(u1)
            slot_kv_dma(u1)
            slot_kproj(u1)
            unit_prologue(u1)
            s_chunk(u1, 0)
            e_chunk(u1, 0)

        for u in range(NU):
            nch = len(st[u]["chunks"])
            for c in range(nch):
                if c + 1 < nch:
                    s_chunk(u, c + 1)
                    e_chunk(u, c + 1)
                    if c == max(0, nch - 2):
                        prefetch_next(u + 1)
                elif u + 1 < NU:
                    prefetch_next(u + 1)
                if c == 0:
                    slot_v_prologue(u)
                av_chunk(u, c)
            epilogue(u)
            if u == max(0, NU // 2 - 1):
                mean_block()

    nc.compile()
    return nc


def _plan(q_len, v_len, B, L, H):
    """Pack unmasked-head jobs into head-slots.

    Returns (struct, assign): struct[s] = tuple of chunk q-widths;
    assign[(core, s)] = (b, h) or None."""
    jobs = []
    for b in range(B):
        nq = min(max(q_len[b], 0), L)
        nh = min(max(v_len[b], 0), H)
        if nq <= 0:
            continue
        r = (nq + NQ - 1) // NQ
        for h in range(nh):
            jobs.append((r, nq, b, h))
    jobs.sort(key=lambda x: (-x[0], -x[1]))
    n_slots = max(1, (len(jobs) + N_CORES - 1) // N_CORES)
    struct = []
    assign = {}
    for s in range(n_slots):
        col = jobs[s * N_CORES:(s + 1) * N_CORES]
        rmax = col[0][0] if col else 1
        widths = []
        for r in range(rmax):
            live = max((min(NQ, nq - r * NQ) for (jr, nq, _, _) in col
                        if r < jr), default=64)
            widths.append(int(live))
        struct.append(tuple(widths))
        for c in range(N_CORES):
            assign[(c, s)] = (col[c][2], col[c][3]) if c < len(col) else None
    return tuple(struct), assign


def kernel(Q_seq, K_seq, V_seq, WQ, WK, WV, Q_len, V_len):
    Q_seq = np.asarray(Q_seq, dtype=np.float32)
    K_seq = np.asarray(K_seq, dtype=np.float32)
    V_seq = np.asarray(V_seq, dtype=np.float32)
    WQ = np.asarray(WQ, dtype=np.float32)
    WK = np.asarray(WK, dtype=np.float32)
    WV = np.asarray(WV, dtype=np.float32)
    q_len = [int(x) for x in np.asarray(Q_len).reshape(-1)]
    v_len = [int(x) for x in np.asarray(V_len).reshape(-1)]
    B, L, d = Q_seq.shape
    H = WQ.shape[1] // d
    scale = 1.0 / math.sqrt(d)

    struct, assign = _plan(q_len, v_len, B, L, H)
    S = len(struct)
    order = _unit_order(struct)
    row_of = {sr: i for i, sr in enumerate(order)}
    NU = len(order)

    if struct not in _cache:
        _cache[struct] = _build(struct)
    nc = _cache[struct]

    # host-side shard prep (transposes, bf16 casts, weight slicing)
    KTb = [np.ascontiguousarray(K_seq[b].T).astype(BF16) for b in range(B)]
    VTb = [np.ascontiguousarray(V_seq[b].T).astype(BF16) for b in range(B)]
    QT = [np.ascontiguousarray(Q_seq[b].T).astype(BF16) for b in range(B)]
    vtb = np.stack([V_seq[b].T for b in range(B)]).astype(np.float32)
    wvm = (WV / float(L)).astype(np.float32)

    in_maps = []
    for c in range(N_CORES):
        qt = np.zeros((NU, 64, NQ), dtype=BF16)
        kt = np.zeros((S, 64, L), dtype=BF16)
        vt = np.zeros((S, 64, L), dtype=BF16)
        w = np.zeros((S, 64, 128), dtype=BF16)
        for s in range(S):
            job = assign[(c, s)]
            if job is None:
                continue
            b, h = job
            kt[s] = KTb[b]
            vt[s] = VTb[b]
            wq_h = WQ[:, h * d:(h + 1) * d]
            wk_h = WK[:, h * d:(h + 1) * d]
            w[s, :, 0:64] = (wk_h @ wq_h.T * scale).astype(BF16)
            w[s, :, 64:128] = WV[:, h * d:(h + 1) * d].astype(BF16)
            for r, nqw in enumerate(struct[s]):
                q0 = min(r * NQ, L - nqw)
                qt[row_of[(s, r)], :, 0:nqw] = QT[b][:, q0:q0 + nqw]
        in_maps.append({"qt": qt, "kt": kt, "vt": vt, "w": w,
                        "vtb": vtb, "wvm": wvm})

    global _last_in_maps
    _last_in_maps = in_maps
    res = run_bass_kernel_spmd(nc, in_maps, core_ids=list(range(N_CORES)))
    results = res.results

    # gather
    out = np.zeros((B, L, H * d), dtype=np.float32)
    mo = results[0]["meanout"]  # [128, 8, B]
    mean_proj = np.transpose(mo, (2, 1, 0)).reshape(B, H * d)  # [B, H*d]
    for b in range(B):
        nq = min(max(q_len[b], 0), L)
        nh = min(max(v_len[b], 0), H)
        if nq > 0 and nh < H:
            out[b, :nq, nh * d:] = mean_proj[b, nh * d:][None, :]
    for (c, s), job in assign.items():
        if job is None:
            continue
        b, h = job
        nq = min(max(q_len[b], 0), L)
        for r, nqw in enumerate(struct[s]):
            q0 = min(r * NQ, L - nqw)
            lo, hi = q0, min(q0 + nqw, nq)
            if hi <= lo:
                continue
            out[b, lo:hi, h * d:(h + 1) * d] = \
                results[c]["out"][row_of[(s, r)], :, :hi - lo].T.astype(np.float32)
    return out


# revision 52
# speedup vs baseline: 127.9839x; 1.0080x over previous
"""Trainium2 Bass kernel for nn_Attention_7919919694519.

Multi-head attention (B=2, L=2048, H=16, d=64) with two data-dependent masks:
  - V_len[b] masks HEADS h >= V_len[b]: the reference adds -1e12 to every
    score of those heads, which collapses (in fp32) to a uniform softmax, so
    the masked head's output is mean_k(v) = (mean_k V_seq) @ WV_h  (rank-1).
  - Q_len[b] zeroes output rows q >= Q_len[b].

Strategy (host-visible Q_len/V_len drive the work list):
  - Only unmasked heads with live q rows do real attention. Each unmasked
    head is a "job" needing ceil(Q_len[b]/512) q-chunks (the last chunk
    trimmed to its live rows). Jobs are packed into head-slots dealt across
    8 NeuronCores (SPMD: same NEFF, different data); slots are interleaved
    round-robin. No collectives; host scatters/gathers.
  - The QK weight product is reassociated: S = Q (WQ WK^T/sqrt(d)) K^T, so
    one per-slot projection ktTilde = (WK_h WQ_h^T/sqrt(d)) @ K^T replaces
    both q- and k-projections; score matmuls read the raw q DMA directly.
  - Per chunk on device: scores S^T[k,q] in bank-aligned PSUM lanes, exp on
    ScalarE (PSUM->SBUF bf16, the bottleneck engine), AV accumulation with
    a ones-column appended to v so softmax denominators fall out of the
    same matmuls, then reciprocal (VectorE) + ones-matmul broadcast +
    multiply, single bf16 DMA out in O^T layout (host transposes during
    gather). Emission is software-pipelined across chunk-units with 3-deep
    score-PSUM buffering so ScalarE never starves.
  - Masked-head rank-1 content: device reduces V_seq over k (VectorE) and
    projects through WV/2048; host broadcasts rows (pure output assembly).
"""

import math
import numpy as np
import ml_dtypes

import concourse.tile as tile
from concourse import bacc, mybir
from concourse.bass_utils import run_bass_kernel_spmd
from contextlib import ExitStack

BF16 = ml_dtypes.bfloat16
N_CORES = 8
B_, L_, D_, H_ = 2, 2048, 64, 16
NQ = 512              # max q rows per chunk
KT = 16               # number of 128-row k tiles (L/128)
SPS_FD = 1536         # score-psum slot free dim (3 banks)

_cache = {}


def _per_bank(nq):
    """k-tiles packed per 512-f32 PSUM bank (power of two so chunks always
    fill whole banks; outputs never cross a bank boundary)."""
    pb = 1
    while pb * 2 <= min(16, 512 // nq):
        pb *= 2
    return pb


def _chunk_plan(nq):
    """k-tiles per score chunk: 2 banks per chunk, 3-deep buffered so score
    matmuls never wait on semaphore latency; narrow q-widths pack several
    k-tiles per bank to keep exp instruction count low."""
    cl = 2 * _per_bank(nq)
    out = [cl] * (KT // cl)
    if KT % cl:
        out = [KT % cl] + out
    return out


def _unit_order(struct):
    """Round-robin (slot, position) order; index = DRAM row in qt/out."""
    order = []
    max_r = max(len(w) for w in struct)
    for r in range(max_r):
        for s in range(len(struct)):
            if r < len(struct[s]):
                order.append((s, r))
    return order


def _build(struct):
    """Build + compile the SPMD NEFF.

    struct: tuple of per-slot tuples of chunk q-widths, e.g.
    ((512, 512, 512, 128), (512, 512, 256))."""
    nc = bacc.Bacc("TRN2", target_bir_lowering=False, debug=False,
                   num_devices=N_CORES)
    dt = mybir.dt
    S = len(struct)
    # interleave slots round-robin so slot prologues overlap earlier slots'
    # compute and the kernel tail lands on the smallest chunk. unit index u
    # equals its DRAM row in qt/out (host uses the same ordering).
    units = [(s, r == 0, struct[s][r]) for s, r in _unit_order(struct)]
    NU = len(units)

    qt_d = nc.dram_tensor("qt", [NU, 64, NQ], dt.bfloat16, kind="ExternalInput").ap()
    kt_d = nc.dram_tensor("kt", [S, 64, L_], dt.bfloat16, kind="ExternalInput").ap()
    vt_d = nc.dram_tensor("vt", [S, 64, L_], dt.bfloat16, kind="ExternalInput").ap()
    w_d = nc.dram_tensor("w", [S, 64, 128], dt.bfloat16, kind="ExternalInput").ap()
    vtb_d = nc.dram_tensor("vtb", [B_, 64, L_], dt.float32, kind="ExternalInput").ap()
    wvm_d = nc.dram_tensor("wvm", [64, H_ * 64], dt.float32, kind="ExternalInput").ap()
    out_d = nc.dram_tensor("out", [NU, 64, NQ], dt.bfloat16, kind="ExternalOutput").ap()
    mo_d = nc.dram_tensor("meanout", [128, 8, B_], dt.float32, kind="ExternalOutput").ap()

    with tile.TileContext(nc) as tc, ExitStack() as ctx:
        sbufs = max(2, S)   # all slots' K/V live concurrently (interleaved)
        inp = ctx.enter_context(tc.tile_pool(name="inp", bufs=sbufs))
        proj = ctx.enter_context(tc.tile_pool(name="proj", bufs=sbufs))
        expp = ctx.enter_context(tc.tile_pool(name="expp", bufs=4))
        ob = ctx.enter_context(tc.tile_pool(name="ob", bufs=4))
        single = ctx.enter_context(tc.tile_pool(name="single", bufs=1))
        ps_s = ctx.enter_context(tc.tile_pool(name="ps_s", bufs=3, space="PSUM"))
        ps_a = ctx.enter_context(tc.tile_pool(name="ps_a", bufs=2, space="PSUM"))

        ones1 = single.tile([1, 64], dt.bfloat16)
        nc.vector.memset(ones1[:], 1.0)

        st = [dict() for _ in range(NU)]
        slot_tiles = {}

        def slot_k_prologue(u):
            # w DMA + tile allocation. The whole QK weight product is folded
            # into the K side: ktTilde = (WK_h WQ_h^T / sqrt(d)) @ K^T once
            # per slot, so per-unit score matmuls read the raw qt DMA with no
            # per-unit projection chain. kt/vt DMAs are issued by slot_kv_dma
            # (after the first unit's qt DMA so the critical path leads the
            # DMA queue); the projection itself runs in slot_kproj.
            s, first, _ = units[u]
            if not first or s in slot_tiles:
                return
            w_sb = inp.tile([64, 128], dt.bfloat16, tag="w", name=f"w{s}")
            nc.gpsimd.dma_start(w_sb[:], w_d[s])
            kt_sb = inp.tile([64, L_], dt.bfloat16, tag="kt", name=f"kt{s}")
            vt_sb = inp.tile([64, L_], dt.bfloat16, tag="vt", name=f"vt{s}")
            slot_tiles[s] = [w_sb, None, None, vt_sb, kt_sb]

        kprojd = set()

        def slot_kproj(u):
            s, first, _ = units[u]
            if not first or s in kprojd:
                return
            kprojd.add(s)
            w_sb, _, _, _, kt_sb = slot_tiles[s]
            ktT = proj.tile([64, L_], dt.bfloat16, tag="ktT", name=f"ktT{s}")
            for j in range(4):
                kps = ps_s.tile([64, 512], dt.float32, tag="ps", name=f"kps{s}_{j}")
                nc.tensor.matmul(kps[:], w_sb[:, 0:64],
                                 kt_sb[:, j * 512:(j + 1) * 512],
                                 start=True, stop=True)
                nc.vector.tensor_copy(ktT[:, j * 512:(j + 1) * 512], kps[:])
            slot_tiles[s][1] = ktT

        kv_dmad = {}

        def slot_kv_dma(u, phase=2):
            s, first, _ = units[u]
            if not first:
                return
            done = kv_dmad.get(s, 0)
            kt_sb, vt_sb = slot_tiles[s][4], slot_tiles[s][3]
            if done < 1 and phase >= 0:
                nc.sync.dma_start(kt_sb[:, 0:512], kt_d[s][:, 0:512])
                kv_dmad[s] = 1
            if kv_dmad[s] < 2 and phase >= 1:
                nc.sync.dma_start(kt_sb[:, 512:], kt_d[s][:, 512:])
                nc.gpsimd.dma_start(vt_sb[:], vt_d[s])
                kv_dmad[s] = 2

        def slot_v_prologue(u):
            s, first, _ = units[u]
            if not first or slot_tiles[s][2] is not None:
                return
            w_sb, vt_sb = slot_tiles[s][0], slot_tiles[s][3]
            # v projection into [k=128, 16, 65] layout (col 64 = ones)
            v_sb = proj.tile([128, KT, 65], dt.bfloat16, tag="v_sb")
            for half in range(2):
                vps = ps_s.tile([128, 8 * 64], dt.float32, tag="ps")
                for j in range(8):
                    t = half * 8 + j
                    nc.tensor.matmul(vps[:, j * 64:(j + 1) * 64],
                                     vt_sb[:, t * 128:(t + 1) * 128],
                                     w_sb[:, 64:128], start=True, stop=True)
                nc.vector.tensor_copy(
                    v_sb[:, half * 8:(half + 1) * 8, 0:64],
                    vps[:].rearrange("p (t d) -> p t d", t=8))
            nc.vector.memset(v_sb[:, :, 64], 1.0)
            slot_tiles[s][2] = v_sb

        def unit_prologue(u):
            s, _, nq = units[u]
            d = st[u]
            d["init"] = True
            d["s"] = s
            d["chunks"] = _chunk_plan(nq)
            d["offs"] = [sum(d["chunks"][:i]) for i in range(len(d["chunks"]) + 1)]
            d["nq"] = nq
            qt_sb = inp.tile([64, nq], dt.bfloat16, tag="qt", name=f"qt{u}")
            nc.sync.dma_start(qt_sb[:], qt_d[u][:, 0:nq])
            d["qTh"] = qt_sb
            d["sps"] = [None] * len(d["chunks"])
            d["ex"] = [None] * len(d["chunks"])

        def s_chunk(u, c):
            d = st[u]
            cl, nq = d["chunks"][c], d["nq"]
            pb = _per_bank(nq)
            nb = (cl + pb - 1) // pb
            sps = ps_s.tile([128, nb, pb, nq], dt.float32, tag="ps",
                            name=f"sps{u}_{c}",
                            padded_shape=[None, None, None, 512 // pb])
            for j in range(cl):
                t = d["offs"][c] + j
                nc.tensor.matmul(sps[:, j // pb, j % pb, :],
                                 slot_tiles[d["s"]][1][:, t * 128:(t + 1) * 128],
                                 d["qTh"][:], start=True, stop=True)
            d["sps"][c] = sps

        def e_chunk(u, c):
            d = st[u]
            cl, nq = d["chunks"][c], d["nq"]
            pb = _per_bank(nq)
            nb = (cl + pb - 1) // pb
            ex = expp.tile([128, nb, pb, nq], dt.bfloat16, tag="ex", name=f"ex{u}_{c}")
            nc.scalar.activation(ex[:], d["sps"][c][:],
                                 mybir.ActivationFunctionType.Exp)
            d["ex"][c] = ex

        def av_chunk(u, c):
            d = st[u]
            nq = d["nq"]
            v_sb = slot_tiles[d["s"]][2]
            if c == 0:
                d["av"] = ps_a.tile([65, nq], dt.float32, tag="pa", name=f"av{u}")
            pb = _per_bank(nq)
            for j in range(d["chunks"][c]):
                t = d["offs"][c] + j
                nc.tensor.matmul(d["av"][:], v_sb[:, t, :],
                                 d["ex"][c][:, j // pb, j % pb, :],
                                 start=(t == 0), stop=(t == KT - 1))

        def epilogue(u):
            # normalize in O^T layout: recip of the sums row, broadcast down
            # 64 partitions via a K=1 matmul, one multiply, one DMA. The
            # final [d, q] -> [q, d] transpose happens on the host (gather).
            d = st[u]
            nq = d["nq"]
            rs = ob.tile([1, nq], dt.bfloat16, tag="rs", name=f"rs{u}")
            with nc.allow_low_precision(reason="softmax denominators are O(1e3); bf16 recip is plenty for the broadcast path"):
                nc.vector.reciprocal(rs[:], d["av"][64:65, :])
            o_sb = ob.tile([64, nq], dt.float32, tag="o_sb", name=f"osb{u}")
            nc.vector.tensor_copy(o_sb[:], d["av"][0:64, :])
            rb = ps_a.tile([64, nq], dt.float32, tag="pa", name=f"rb{u}")
            nc.tensor.matmul(rb[:], ones1[:, 0:64], rs[:], start=True, stop=True)
            ot = ob.tile([64, nq], dt.bfloat16, tag="ot", name=f"ot{u}")
            with nc.allow_low_precision(reason="final output cast; 2e-2 rel-err budget"):
                nc.vector.tensor_mul(ot[:], o_sb[:], rb[:])
            nc.sync.dma_start(out_d[u][:, 0:nq], ot[:])
            st[u].clear()

        def mean_block():
            # masked-head rank-1 content: (sum_k V_seq) @ (WV/2048)
            wvm_sb = single.tile([64, H_ * 64], dt.float32)
            nc.sync.dma_start(wvm_sb[:], wvm_d[:])
            mvt = single.tile([64, B_], dt.float32)
            mvt4 = single.tile([64, B_, 4], dt.float32)
            for b in range(B_):
                vtb_sb = inp.tile([64, L_], dt.float32, tag="vtb")
                nc.sync.dma_start(vtb_sb[:], vtb_d[b])
                for j in range(4):
                    nc.vector.reduce_sum(mvt4[:, b, j:j + 1],
                                         vtb_sb[:, j * 512:(j + 1) * 512],
                                         axis=mybir.AxisListType.X)
                nc.vector.reduce_sum(mvt[:, b:b + 1], mvt4[:, b, :],
                                     axis=mybir.AxisListType.X)
            mo_sb = single.tile([128, 8, B_], dt.float32)
            mps = ps_a.tile([128, 8, B_], dt.float32, tag="pa", name="mps")
            for c in range(8):
                nc.tensor.matmul(mps[:, c, :], wvm_sb[:, c * 128:(c + 1) * 128],
                                 mvt[:], start=True, stop=True)
            nc.vector.tensor_copy(mo_sb[:], mps[:])
            nc.sync.dma_start(mo_d[:], mo_sb[:])

        # software pipeline across chunk-units: the next unit's prologue and
        # first score chunk are emitted before this unit's AV tail/epilogue so
        # ScalarE never starves at unit boundaries.
        slot_k_prologue(0)
        slot_kv_dma(0, phase=0)
        unit_prologue(0)
        slot_kv_dma(0, phase=1)
        slot_kproj(0)
        s_chunk(0, 0)
        e_chunk(0, 0)
        # prefetch every other slot's K/V DMAs + projection while unit 0 runs
        first_unit = {}
        for i, (s, first, _) in enumerate(units):
            if first:
                first_unit[s] = i
        for s in range(1, S):
            slot_k_prologue(first_unit[s])
            slot_kv_dma(first_unit[s])
            slot_kproj(first_unit[s])
        def prefetch_next(u1):
            if u1 >= NU or st[u1].get("init"):
                return
            slot_k_prologue(u1)
            slot_kv_dma(u1)
            slot_kproj(u1)
            unit_prologue(u1)
            s_chunk(u1, 0)
            e_chunk(u1, 0)

        for u in range(NU):
            nch = len(st[u]["chunks"])
            for c in range(nch):
                if c + 1 < nch:
                    s_chunk(u, c + 1)
                    e_chunk(u, c + 1)
                    if c == max(0, nch - 2):
                        prefetch_next(u + 1)
                elif u + 1 < NU:
                    prefetch_next(u + 1)
                if c == 0:
                    slot_v_prologue(u)
                av_chunk(u, c)
            epilogue(u)
            if u == max(0, NU // 2 - 1):
                mean_block()

    nc.compile()
    return nc


def _plan(q_len, v_len, B, L, H):
    """Pack unmasked-head jobs into head-slots.

    Returns (struct, assign): struct[s] = tuple of chunk q-widths;
    assign[(core, s)] = (b, h) or None."""
    jobs = []
    for b in range(B):
        nq = min(max(q_len[b], 0), L)
        nh = min(max(v_len[b], 0), H)
        if nq <= 0:
            continue
        r = (nq + NQ - 1) // NQ
        for h in range(nh):
            jobs.append((r, nq, b, h))
    jobs.sort(key=lambda x: (-x[0], -x[1]))
    n_slots = max(1, (len(jobs) + N_CORES - 1) // N_CORES)
    struct = []
    assign = {}
    for s in range(n_slots):
        col = jobs[s * N_CORES:(s + 1) * N_CORES]
        rmax = col[0][0] if col else 1
        widths = []
        for r in range(rmax):
            live = max((min(NQ, nq - r * NQ) for (jr, nq, _, _) in col
                        if r < jr), default=64)
            widths.append(int(live))
        struct.append(tuple(widths))
        for c in range(N_CORES):
            assign[(c, s)] = (col[c][2], col[c][3]) if c < len(col) else None
    return tuple(struct), assign


def kernel(Q_seq, K_seq, V_seq, WQ, WK, WV, Q_len, V_len):
    Q_seq = np.asarray(Q_seq, dtype=np.float32)
    K_seq = np.asarray(K_seq, dtype=np.float32)
    V_seq = np.asarray(V_seq, dtype=np.float32)
    WQ = np.asarray(WQ, dtype=np.float32)
    WK = np.asarray(WK, dtype=np.float32)
    WV = np.asarray(WV, dtype=np.float32)
    q_len = [int(x) for x in np.asarray(Q_len).reshape(-1)]
    v_len = [int(x) for x in np.asarray(V_len).reshape(-1)]
    B, L, d = Q_seq.shape
    H = WQ.shape[1] // d
    scale = 1.0 / math.sqrt(d)

    struct, assign = _plan(q_len, v_len, B, L, H)
    S = len(struct)
    order = _unit_order(struct)
    row_of = {sr: i for i, sr in enumerate(order)}
    NU = len(order)

    if struct not in _cache:
        _cache[struct] = _build(struct)
    nc = _cache[struct]

    # host-side shard prep (transposes, bf16 casts, weight slicing)
    KTb = [np.ascontiguousarray(K_seq[b].T).astype(BF16) for b in range(B)]
    VTb = [np.ascontiguousarray(V_seq[b].T).astype(BF16) for b in range(B)]
    QT = [np.ascontiguousarray(Q_seq[b].T).astype(BF16) for b in range(B)]
    vtb = np.stack([V_seq[b].T for b in range(B)]).astype(np.float32)
    wvm = (WV / float(L)).astype(np.float32)

    in_maps = []
    for c in range(N_CORES):
        qt = np.zeros((NU, 64, NQ), dtype=BF16)
        kt = np.zeros((S, 64, L), dtype=BF16)
        vt = np.zeros((S, 64, L), dtype=BF16)
        w = np.zeros((S, 64, 128), dtype=BF16)
        for s in range(S):
            job = assign[(c, s)]
            if job is None:
                continue
            b, h = job
            kt[s] = KTb[b]
            vt[s] = VTb[b]
            wq_h = WQ[:, h * d:(h + 1) * d]
            wk_h = WK[:, h * d:(h + 1) * d]
            w[s, :, 0:64] = (wk_h @ wq_h.T * scale).astype(BF16)
            w[s, :, 64:128] = WV[:, h * d:(h + 1) * d].astype(BF16)
            for r, nqw in enumerate(struct[s]):
                q0 = min(r * NQ, L - nqw)
                qt[row_of[(s, r)], :, 0:nqw] = QT[b][:, q0:q0 + nqw]
        in_maps.append({"qt": qt, "kt": kt, "vt": vt, "w": w,
                        "vtb": vtb, "wvm": wvm})

    global _last_in_maps
    _last_in_maps = in_maps
    res = run_bass_kernel_spmd(nc, in_maps, core_ids=list(range(N_CORES)))
    results = res.results

    # gather
    out = np.zeros((B, L, H * d), dtype=np.float32)
    mo = results[0]["meanout"]  # [128, 8, B]
    mean_proj = np.transpose(mo, (2, 1, 0)).reshape(B, H * d)  # [B, H*d]
    for b in range(B):
        nq = min(max(q_len[b], 0), L)
        nh = min(max(v_len[b], 0), H)
        if nq > 0 and nh < H:
            out[b, :nq, nh * d:] = mean_proj[b, nh * d:][None, :]
    for (c, s), job in assign.items():
        if job is None:
            continue
        b, h = job
        nq = min(max(q_len[b], 0), L)
        for r, nqw in enumerate(struct[s]):
            q0 = min(r * NQ, L - nqw)
            lo, hi = q0, min(q0 + nqw, nq)
            if hi <= lo:
                continue
            out[b, lo:hi, h * d:(h + 1) * d] = \
                results[c]["out"][row_of[(s, r)], :, :hi - lo].T.astype(np.float32)
    return out


# revision 55
# speedup vs baseline: 128.4816x; 1.0039x over previous
"""Trainium2 Bass kernel for nn_Attention_7919919694519.

Multi-head attention (B=2, L=2048, H=16, d=64) with two data-dependent masks:
  - V_len[b] masks HEADS h >= V_len[b]: the reference adds -1e12 to every
    score of those heads, which collapses (in fp32) to a uniform softmax, so
    the masked head's output is mean_k(v) = (mean_k V_seq) @ WV_h  (rank-1).
  - Q_len[b] zeroes output rows q >= Q_len[b].

Strategy (host-visible Q_len/V_len drive the work list):
  - Only unmasked heads with live q rows do real attention. Each unmasked
    head is a "job" needing ceil(Q_len[b]/512) q-chunks (the last chunk
    trimmed to its live rows). Jobs are packed into head-slots dealt across
    8 NeuronCores (SPMD: same NEFF, different data); slots are interleaved
    round-robin. No collectives; host scatters/gathers.
  - The QK weight product is reassociated: S = Q (WQ WK^T/sqrt(d)) K^T, so
    one per-slot projection ktTilde = (WK_h WQ_h^T/sqrt(d)) @ K^T replaces
    both q- and k-projections; score matmuls read the raw q DMA directly.
  - Per chunk on device: scores S^T[k,q] in bank-aligned PSUM lanes, exp on
    ScalarE (PSUM->SBUF bf16, the bottleneck engine), AV accumulation with
    a ones-column appended to v so softmax denominators fall out of the
    same matmuls, then reciprocal (VectorE) + ones-matmul broadcast +
    multiply, single bf16 DMA out in O^T layout (host transposes during
    gather). Emission is software-pipelined across chunk-units with 3-deep
    score-PSUM buffering so ScalarE never starves.
  - Masked-head rank-1 content: device reduces V_seq over k (VectorE) and
    projects through WV/2048; host broadcasts rows (pure output assembly).
"""

import math
import numpy as np
import ml_dtypes

import concourse.tile as tile
from concourse import bacc, mybir
from concourse.bass_utils import run_bass_kernel_spmd
from contextlib import ExitStack

BF16 = ml_dtypes.bfloat16
N_CORES = 8
B_, L_, D_, H_ = 2, 2048, 64, 16
NQ = 512              # max q rows per chunk
KT = 16               # number of 128-row k tiles (L/128)
SPS_FD = 1536         # score-psum slot free dim (3 banks)

_cache = {}


def _per_bank(nq):
    """k-tiles packed per 512-f32 PSUM bank (power of two so chunks always
    fill whole banks; outputs never cross a bank boundary)."""
    pb = 1
    while pb * 2 <= min(16, 512 // nq):
        pb *= 2
    return pb


def _chunk_plan(nq):
    """k-tiles per score chunk: 2 banks per chunk, 3-deep buffered so score
    matmuls never wait on semaphore latency; narrow q-widths pack several
    k-tiles per bank to keep exp instruction count low."""
    cl = 2 * _per_bank(nq)
    out = [cl] * (KT // cl)
    if KT % cl:
        out = [KT % cl] + out
    return out


def _unit_order(struct):
    """Round-robin (slot, position) order; index = DRAM row in qt/out."""
    order = []
    max_r = max(len(w) for w in struct)
    for r in range(max_r):
        for s in range(len(struct)):
            if r < len(struct[s]):
                order.append((s, r))
    return order


def _build(struct):
    """Build + compile the SPMD NEFF.

    struct: tuple of per-slot tuples of chunk q-widths, e.g.
    ((512, 512, 512, 128), (512, 512, 256))."""
    nc = bacc.Bacc("TRN2", target_bir_lowering=False, debug=False,
                   num_devices=N_CORES)
    dt = mybir.dt
    S = len(struct)
    # interleave slots round-robin so slot prologues overlap earlier slots'
    # compute and the kernel tail lands on the smallest chunk. unit index u
    # equals its DRAM row in qt/out (host uses the same ordering).
    units = [(s, r == 0, struct[s][r]) for s, r in _unit_order(struct)]
    NU = len(units)

    qt_d = nc.dram_tensor("qt", [NU, 64, NQ], dt.bfloat16, kind="ExternalInput").ap()
    kt_d = nc.dram_tensor("kt", [S, 64, L_], dt.bfloat16, kind="ExternalInput").ap()
    vt_d = nc.dram_tensor("vt", [S, 64, L_], dt.bfloat16, kind="ExternalInput").ap()
    w_d = nc.dram_tensor("w", [S, 64, 128], dt.bfloat16, kind="ExternalInput").ap()
    vtb_d = nc.dram_tensor("vtb", [B_, 64, L_], dt.float32, kind="ExternalInput").ap()
    wvm_d = nc.dram_tensor("wvm", [64, H_ * 64], dt.float32, kind="ExternalInput").ap()
    out_d = nc.dram_tensor("out", [NU, 64, NQ], dt.bfloat16, kind="ExternalOutput").ap()
    mo_d = nc.dram_tensor("meanout", [128, 8, B_], dt.float32, kind="ExternalOutput").ap()

    with tile.TileContext(nc) as tc, ExitStack() as ctx:
        sbufs = max(2, S)   # all slots' K/V live concurrently (interleaved)
        inp = ctx.enter_context(tc.tile_pool(name="inp", bufs=sbufs))
        proj = ctx.enter_context(tc.tile_pool(name="proj", bufs=sbufs))
        expp = ctx.enter_context(tc.tile_pool(name="expp", bufs=4))
        ob = ctx.enter_context(tc.tile_pool(name="ob", bufs=4))
        single = ctx.enter_context(tc.tile_pool(name="single", bufs=1))
        ps_s = ctx.enter_context(tc.tile_pool(name="ps_s", bufs=3, space="PSUM"))
        ps_a = ctx.enter_context(tc.tile_pool(name="ps_a", bufs=2, space="PSUM"))

        ones1 = single.tile([1, 64], dt.bfloat16)
        nc.vector.memset(ones1[:], 1.0)

        st = [dict() for _ in range(NU)]
        slot_tiles = {}

        def slot_k_prologue(u):
            # w DMA + tile allocation. The whole QK weight product is folded
            # into the K side: ktTilde = (WK_h WQ_h^T / sqrt(d)) @ K^T once
            # per slot, so per-unit score matmuls read the raw qt DMA with no
            # per-unit projection chain. kt/vt DMAs are issued by slot_kv_dma
            # (after the first unit's qt DMA so the critical path leads the
            # DMA queue); the projection itself runs in slot_kproj.
            s, first, _ = units[u]
            if not first or s in slot_tiles:
                return
            w_sb = inp.tile([64, 128], dt.bfloat16, tag="w", name=f"w{s}")
            nc.gpsimd.dma_start(w_sb[:], w_d[s])
            kt_sb = inp.tile([64, L_], dt.bfloat16, tag="kt", name=f"kt{s}")
            vt_sb = inp.tile([64, L_], dt.bfloat16, tag="vt", name=f"vt{s}")
            slot_tiles[s] = [w_sb, None, None, vt_sb, kt_sb]

        kprojd = set()

        def slot_kproj(u):
            s, first, _ = units[u]
            if not first or s in kprojd:
                return
            kprojd.add(s)
            w_sb, _, _, _, kt_sb = slot_tiles[s]
            ktT = proj.tile([64, L_], dt.bfloat16, tag="ktT", name=f"ktT{s}")
            for j in range(4):
                kps = ps_s.tile([64, 512], dt.float32, tag="ps", name=f"kps{s}_{j}")
                nc.tensor.matmul(kps[:], w_sb[:, 0:64],
                                 kt_sb[:, j * 512:(j + 1) * 512],
                                 start=True, stop=True)
                if j == 0:
                    nc.scalar.copy(ktT[:, j * 512:(j + 1) * 512], kps[:])
                else:
                    nc.vector.tensor_copy(ktT[:, j * 512:(j + 1) * 512], kps[:])
            slot_tiles[s][1] = ktT

        kv_dmad = {}

        def slot_kv_dma(u, phase=2):
            s, first, _ = units[u]
            if not first:
                return
            done = kv_dmad.get(s, 0)
            kt_sb, vt_sb = slot_tiles[s][4], slot_tiles[s][3]
            if done < 1 and phase >= 0:
                nc.sync.dma_start(kt_sb[:, 0:512], kt_d[s][:, 0:512])
                kv_dmad[s] = 1
            if kv_dmad[s] < 2 and phase >= 1:
                nc.sync.dma_start(kt_sb[:, 512:], kt_d[s][:, 512:])
                nc.gpsimd.dma_start(vt_sb[:], vt_d[s])
                kv_dmad[s] = 2

        def slot_v_prologue(u):
            s, first, _ = units[u]
            if not first or slot_tiles[s][2] is not None:
                return
            w_sb, vt_sb = slot_tiles[s][0], slot_tiles[s][3]
            # v projection into [k=128, 16, 65] layout (col 64 = ones)
            v_sb = proj.tile([128, KT, 65], dt.bfloat16, tag="v_sb")
            for half in range(2):
                vps = ps_s.tile([128, 8 * 64], dt.float32, tag="ps")
                for j in range(8):
                    t = half * 8 + j
                    nc.tensor.matmul(vps[:, j * 64:(j + 1) * 64],
                                     vt_sb[:, t * 128:(t + 1) * 128],
                                     w_sb[:, 64:128], start=True, stop=True)
                nc.vector.tensor_copy(
                    v_sb[:, half * 8:(half + 1) * 8, 0:64],
                    vps[:].rearrange("p (t d) -> p t d", t=8))
            nc.vector.memset(v_sb[:, :, 64], 1.0)
            slot_tiles[s][2] = v_sb

        def unit_prologue(u):
            s, _, nq = units[u]
            d = st[u]
            d["init"] = True
            d["s"] = s
            d["chunks"] = _chunk_plan(nq)
            d["offs"] = [sum(d["chunks"][:i]) for i in range(len(d["chunks"]) + 1)]
            d["nq"] = nq
            qt_sb = inp.tile([64, nq], dt.bfloat16, tag="qt", name=f"qt{u}")
            nc.sync.dma_start(qt_sb[:], qt_d[u][:, 0:nq])
            d["qTh"] = qt_sb
            d["sps"] = [None] * len(d["chunks"])
            d["ex"] = [None] * len(d["chunks"])

        def s_chunk(u, c):
            d = st[u]
            cl, nq = d["chunks"][c], d["nq"]
            pb = _per_bank(nq)
            nb = (cl + pb - 1) // pb
            sps = ps_s.tile([128, nb, pb, nq], dt.float32, tag="ps",
                            name=f"sps{u}_{c}",
                            padded_shape=[None, None, None, 512 // pb])
            for j in range(cl):
                t = d["offs"][c] + j
                nc.tensor.matmul(sps[:, j // pb, j % pb, :],
                                 slot_tiles[d["s"]][1][:, t * 128:(t + 1) * 128],
                                 d["qTh"][:], start=True, stop=True)
            d["sps"][c] = sps

        def e_chunk(u, c):
            d = st[u]
            cl, nq = d["chunks"][c], d["nq"]
            pb = _per_bank(nq)
            nb = (cl + pb - 1) // pb
            ex = expp.tile([128, nb, pb, nq], dt.bfloat16, tag="ex", name=f"ex{u}_{c}")
            nc.scalar.activation(ex[:], d["sps"][c][:],
                                 mybir.ActivationFunctionType.Exp)
            d["ex"][c] = ex

        def av_chunk(u, c):
            d = st[u]
            nq = d["nq"]
            v_sb = slot_tiles[d["s"]][2]
            if c == 0:
                d["av"] = ps_a.tile([65, nq], dt.float32, tag="pa", name=f"av{u}")
            pb = _per_bank(nq)
            for j in range(d["chunks"][c]):
                t = d["offs"][c] + j
                nc.tensor.matmul(d["av"][:], v_sb[:, t, :],
                                 d["ex"][c][:, j // pb, j % pb, :],
                                 start=(t == 0), stop=(t == KT - 1))

        def epilogue(u):
            # normalize in O^T layout: recip of the sums row, broadcast down
            # 64 partitions via a K=1 matmul, one multiply, one DMA. The
            # final [d, q] -> [q, d] transpose happens on the host (gather).
            d = st[u]
            nq = d["nq"]
            rs = ob.tile([1, nq], dt.bfloat16, tag="rs", name=f"rs{u}")
            with nc.allow_low_precision(reason="softmax denominators are O(1e3); bf16 recip is plenty for the broadcast path"):
                nc.vector.reciprocal(rs[:], d["av"][64:65, :])
            o_sb = ob.tile([64, nq], dt.float32, tag="o_sb", name=f"osb{u}")
            nc.vector.tensor_copy(o_sb[:], d["av"][0:64, :])
            rb = ps_a.tile([64, nq], dt.float32, tag="pa", name=f"rb{u}")
            nc.tensor.matmul(rb[:], ones1[:, 0:64], rs[:], start=True, stop=True)
            ot = ob.tile([64, nq], dt.bfloat16, tag="ot", name=f"ot{u}")
            with nc.allow_low_precision(reason="final output cast; 2e-2 rel-err budget"):
                nc.vector.tensor_mul(ot[:], o_sb[:], rb[:])
            nc.sync.dma_start(out_d[u][:, 0:nq], ot[:])
            st[u].clear()

        def mean_block():
            # masked-head rank-1 content: (sum_k V_seq) @ (WV/2048)
            wvm_sb = single.tile([64, H_ * 64], dt.float32)
            nc.sync.dma_start(wvm_sb[:], wvm_d[:])
            mvt = single.tile([64, B_], dt.float32)
            mvt4 = single.tile([64, B_, 4], dt.float32)
            for b in range(B_):
                vtb_sb = inp.tile([64, L_], dt.float32, tag="vtb")
                nc.sync.dma_start(vtb_sb[:], vtb_d[b])
                for j in range(4):
                    nc.vector.reduce_sum(mvt4[:, b, j:j + 1],
                                         vtb_sb[:, j * 512:(j + 1) * 512],
                                         axis=mybir.AxisListType.X)
                nc.vector.reduce_sum(mvt[:, b:b + 1], mvt4[:, b, :],
                                     axis=mybir.AxisListType.X)
            mo_sb = single.tile([128, 8, B_], dt.float32)
            mps = ps_a.tile([128, 8, B_], dt.float32, tag="pa", name="mps")
            for c in range(8):
                nc.tensor.matmul(mps[:, c, :], wvm_sb[:, c * 128:(c + 1) * 128],
                                 mvt[:], start=True, stop=True)
            nc.vector.tensor_copy(mo_sb[:], mps[:])
            nc.sync.dma_start(mo_d[:], mo_sb[:])

        # software pipeline across chunk-units: the next unit's prologue and
        # first score chunk are emitted before this unit's AV tail/epilogue so
        # ScalarE never starves at unit boundaries.
        slot_k_prologue(0)
        slot_kv_dma(0, phase=0)
        unit_prologue(0)
        slot_kv_dma(0, phase=1)
        slot_kproj(0)
        s_chunk(0, 0)
        e_chunk(0, 0)
        # prefetch every other slot's K/V DMAs + projection while unit 0 runs
        first_unit = {}
        for i, (s, first, _) in enumerate(units):
            if first:
                first_unit[s] = i
        for s in range(1, S):
            slot_k_prologue(first_unit[s])
            slot_kv_dma(first_unit[s])
            slot_kproj(first_unit[s])
        def prefetch_next(u1):
            if u1 >= NU or st[u1].get("init"):
                return
            slot_k_prologue(u1)
            slot_kv_dma(u1)
            slot_kproj(u1)
            unit_prologue(u1)
            s_chunk(u1, 0)
            e_chunk(u1, 0)

        for u in range(NU):
            nch = len(st[u]["chunks"])
            for c in range(nch):
                if c + 1 < nch:
                    s_chunk(u, c + 1)
                    e_chunk(u, c + 1)
                    if c == max(0, nch - 2):
                        prefetch_next(u + 1)
                elif u + 1 < NU:
                    prefetch_next(u + 1)
                if c == 0:
                    slot_v_prologue(u)
                av_chunk(u, c)
            epilogue(u)
            if u == max(0, NU // 2 - 1):
                mean_block()

    nc.compile()
    return nc


def _plan(q_len, v_len, B, L, H):
    """Pack unmasked-head jobs into head-slots.

    Returns (struct, assign): struct[s] = tuple of chunk q-widths;
    assign[(core, s)] = (b, h) or None."""
    jobs = []
    for b in range(B):
        nq = min(max(q_len[b], 0), L)
        nh = min(max(v_len[b], 0), H)
        if nq <= 0:
            continue
        r = (nq + NQ - 1) // NQ
        for h in range(nh):
            jobs.append((r, nq, b, h))
    jobs.sort(key=lambda x: (-x[0], -x[1]))
    n_slots = max(1, (len(jobs) + N_CORES - 1) // N_CORES)
    struct = []
    assign = {}
    for s in range(n_slots):
        col = jobs[s * N_CORES:(s + 1) * N_CORES]
        rmax = col[0][0] if col else 1
        widths = []
        for r in range(rmax):
            live = max((min(NQ, nq - r * NQ) for (jr, nq, _, _) in col
                        if r < jr), default=64)
            widths.append(int(live))
        struct.append(tuple(widths))
        for c in range(N_CORES):
            assign[(c, s)] = (col[c][2], col[c][3]) if c < len(col) else None
    return tuple(struct), assign


def kernel(Q_seq, K_seq, V_seq, WQ, WK, WV, Q_len, V_len):
    Q_seq = np.asarray(Q_seq, dtype=np.float32)
    K_seq = np.asarray(K_seq, dtype=np.float32)
    V_seq = np.asarray(V_seq, dtype=np.float32)
    WQ = np.asarray(WQ, dtype=np.float32)
    WK = np.asarray(WK, dtype=np.float32)
    WV = np.asarray(WV, dtype=np.float32)
    q_len = [int(x) for x in np.asarray(Q_len).reshape(-1)]
    v_len = [int(x) for x in np.asarray(V_len).reshape(-1)]
    B, L, d = Q_seq.shape
    H = WQ.shape[1] // d
    scale = 1.0 / math.sqrt(d)

    struct, assign = _plan(q_len, v_len, B, L, H)
    S = len(struct)
    order = _unit_order(struct)
    row_of = {sr: i for i, sr in enumerate(order)}
    NU = len(order)

    if struct not in _cache:
        _cache[struct] = _build(struct)
    nc = _cache[struct]

    # host-side shard prep (transposes, bf16 casts, weight slicing)
    KTb = [np.ascontiguousarray(K_seq[b].T).astype(BF16) for b in range(B)]
    VTb = [np.ascontiguousarray(V_seq[b].T).astype(BF16) for b in range(B)]
    QT = [np.ascontiguousarray(Q_seq[b].T).astype(BF16) for b in range(B)]
    vtb = np.stack([V_seq[b].T for b in range(B)]).astype(np.float32)
    wvm = (WV / float(L)).astype(np.float32)

    in_maps = []
    for c in range(N_CORES):
        qt = np.zeros((NU, 64, NQ), dtype=BF16)
        kt = np.zeros((S, 64, L), dtype=BF16)
        vt = np.zeros((S, 64, L), dtype=BF16)
        w = np.zeros((S, 64, 128), dtype=BF16)
        for s in range(S):
            job = assign[(c, s)]
            if job is None:
                continue
            b, h = job
            kt[s] = KTb[b]
            vt[s] = VTb[b]
            wq_h = WQ[:, h * d:(h + 1) * d]
            wk_h = WK[:, h * d:(h + 1) * d]
            w[s, :, 0:64] = (wk_h @ wq_h.T * scale).astype(BF16)
            w[s, :, 64:128] = WV[:, h * d:(h + 1) * d].astype(BF16)
            for r, nqw in enumerate(struct[s]):
                q0 = min(r * NQ, L - nqw)
                qt[row_of[(s, r)], :, 0:nqw] = QT[b][:, q0:q0 + nqw]
        in_maps.append({"qt": qt, "kt": kt, "vt": vt, "w": w,
                        "vtb": vtb, "wvm": wvm})

    global _last_in_maps
    _last_in_maps = in_maps
    res = run_bass_kernel_spmd(nc, in_maps, core_ids=list(range(N_CORES)))
    results = res.results

    # gather
    out = np.zeros((B, L, H * d), dtype=np.float32)
    mo = results[0]["meanout"]  # [128, 8, B]
    mean_proj = np.transpose(mo, (2, 1, 0)).reshape(B, H * d)  # [B, H*d]
    for b in range(B):
        nq = min(max(q_len[b], 0), L)
        nh = min(max(v_len[b], 0), H)
        if nq > 0 and nh < H:
            out[b, :nq, nh * d:] = mean_proj[b, nh * d:][None, :]
    for (c, s), job in assign.items():
        if job is None:
            continue
        b, h = job
        nq = min(max(q_len[b], 0), L)
        for r, nqw in enumerate(struct[s]):
            q0 = min(r * NQ, L - nqw)
            lo, hi = q0, min(q0 + nqw, nq)
            if hi <= lo:
                continue
            out[b, lo:hi, h * d:(h + 1) * d] = \
                results[c]["out"][row_of[(s, r)], :, :hi - lo].T.astype(np.float32)
    return out
